# revision 1
# baseline (speedup 1.0000x reference)
"""Point-Transformer block as a Bass/Tile kernel for 8 Trainium2 NeuronCores.

Strategy
--------
Data-parallel over batch: core b handles batch element b (B == 8 == n_cores).

Host-side algebraic folding (all on 64x64-ish weights, negligible cost):
  * x1/x2 are never materialized: the gate-logit path folds into per-j
    64->8 matmuls with weights D_j = -Ww1_j@W2 (+ Ww1-rowsum@W1 for j==0).
  * pt_conv collapses: ptsn >= 0 so relu(Wp1*ptsn) = relu(Wp1)*ptsn, hence
    ptf = v (x) ptsn with v = Wp2 @ relu(Wp1); its contribution to the
    gated sum is computed in 80-row (group,k) space and folded into two
    `vsel` broadcast matmuls whose contraction performs the k-sum.
  * b3 is deferred through the softmax (sum_k ws = 1); bout rides a
    ones-row matmul inside the output-head PSUM accumulation.

Layout: 512-point tiles, the two 256-point halves packed on partitions
[0:64)/[64:128) so all 64-channel matmuls run with block-diagonal weights
at full PE occupancy.  feats are bf16 and k-major in HBM so xn PSUM
chunks are exactly [128, 512]; activations/weights are bf16 (matmul cols
then cost 1 PE cycle); the gate weights e are fp8e4 (the softmax
normalization cancels most of the quantization).

The per-channel gate weights ws[c,k,n] = e[(c%8)*10+k,n] are a pure
partition replication of e: engines cannot replicate partitions, so e is
staged to an HBM scratch buffer and DMA-gathered back in the replicated
[128,10,256] layout (Pool/SWDGE queue).  This removes both the duplicate
128-row exp from Act and the per-k wkat matmuls from PE.

Software pipeline (per iteration): consume tile it (x3 matmuls + DVE
gate products + Pool k-reduction tree), build tile it+2's xn and launch
its gate chain (G -> h -> e -> exp -> HBM round trip) so the ~9us chain
latency hides under two tile periods; the normalize/relu/head tail of
tile it-1 is emitted after tile it's mults so its Pool-tree wait never
head-blocks DVE's queue; rsb/P PSUM banks are parity double-buffered.
GPSIMD never touches PSUM (illegal on HW); matmul PSUM writes start at
partition 0/32/64 only; DVE instructions read at most one PSUM operand.

I/O: ptsn ships unreplicated and is expanded on-device by a
partition-broadcast load DMA; the output is written bf16 (host upcasts
after the gather); scratch DMA issue is split across the Pool/SWDGE and
SP/HWDGE queues.  Pipeline fill is overlapped twice: the prologue xn
relus alternate Act/DVE (DVE is idle until the first gate weights
arrive), and the two prologue ws gathers deliver their early k-slices
in a separate DMA so tile 0's gate mults start sooner.
"""

import numpy as np

B, N, K = 8, 8192, 10
CH = 64          # IN == MID == OUT
SP = 8
GN = CH // SP    # 8 gate channels
TN = 512         # points per tile
TN2 = TN // 2    # points per partition-half
NT = N // TN     # 16 tiles

# packed bf16 const layout: name -> (rows, cols, col offset)
_BF16_LAYOUT = {}
_off = 0
for _name, _r, _c in [
    ("w0ddT", 128, 128), ("w3ddT", 128, 128), ("dls", 128, 160),
    ("w1vA", 10, 16), ("w1vB", 10, 16),
    ("ww2A", 16, 80), ("ww2B", 16, 80), ("s8selA", 80, 16),
    ("vselA", 80, 128), ("vselB", 80, 128),
    ("woutddT", 128, 128), ("idd", 128, 128), ("boutrow", 1, 128),
    ("ones1", 1, 256), ("obcb", 16, 128),
]:
    _BF16_LAYOUT[_name] = (_r, _c, _off)
    _off += _c
_BF16_W = _off

_F8_LAYOUT = {}
_off = 0
for _name, _r, _c in [
    ("s8selA", 80, 16), ("s8selB", 80, 16),
]:
    _F8_LAYOUT[_name] = (_r, _c, _off)
    _off += _c
_F8_W = _off

_F32_LAYOUT = {}
_off = 0
for _name, _r, _c in [
    ("b0dd", 128, 1), ("cbdd", 16, 1), ("b3dd", 128, 1),
    ("bw2col", 80, 1), ("boutdd", 128, 1),
]:
    _F32_LAYOUT[_name] = (_r, _c, _off)
    _off += _c
_F32_W = _off

_CACHE = {}


def _build_bass():
    import concourse.bacc as bacc
    import concourse.tile as tile
    from concourse import mybir

    f32 = mybir.dt.float32
    f32r = mybir.dt.float32r
    bf16 = mybir.dt.bfloat16
    f8 = mybir.dt.float8e4
    AF = mybir.ActivationFunctionType
    OP = mybir.AluOpType

    nc = bacc.Bacc("TRN2", target_bir_lowering=False)

    def mm(out, lhsT, rhs, **kw):
        nc.tensor.matmul(out, lhsT, rhs, **kw)

    # ---------------- DRAM I/O ----------------
    # feats pre-packed on host: [c + 64*half, ((tile, k), within)] k-major
    feats_d = nc.dram_tensor("feats", [128, N * K // 2], bf16,
                             kind="ExternalInput")
    # HBM staging for the gate-weight broadcast (written and re-read by
    # the kernel, double-buffered across tiles): [it%2, half, 80, n]
    e_scr = nc.dram_tensor("e_scr", [3, 2, 80, TN2], f8,
                            kind="Internal")
    # ptsn per tile, [tile, k, h*256+n]; the 8-group replication happens
    # in the load DMA via a partition-broadcast access pattern
    pt80_d = nc.dram_tensor("pt80", [NT, K, 2 * TN2], bf16,
                            kind="ExternalInput")
    cpack_b_d = nc.dram_tensor("cpack_b", [128, _BF16_W], bf16,
                               kind="ExternalInput")
    cpack_f_d = nc.dram_tensor("cpack_f", [128, _F32_W], f32,
                               kind="ExternalInput")
    cpack_8_d = nc.dram_tensor("cpack_8", [128, _F8_W], f8,
                               kind="ExternalInput")
    out_d = nc.dram_tensor("out", [128, N // 2], bf16,
                           kind="ExternalOutput")

    with tile.TileContext(nc) as tc:
        with (
            tc.tile_pool(name="singles", bufs=1) as singles,
            tc.tile_pool(name="bigio", bufs=3) as bigio,
            tc.tile_pool(name="xnpool", bufs=4) as xnpool,
            tc.tile_pool(name="p80pool", bufs=4) as p80pool,
            tc.tile_pool(name="work", bufs=3) as work,
            tc.tile_pool(name="small", bufs=3) as small,
            tc.tile_pool(name="xnps", bufs=2, space="PSUM") as xnps_pool,
            tc.tile_pool(name="x3ps", bufs=3, space="PSUM") as x3ps_pool,
            tc.tile_pool(name="sbank", bufs=1, space="PSUM") as sbank_pool,
        ):
            # ---- packed consts in SBUF (two DMAs) ----
            cpack_b = singles.tile([128, _BF16_W], bf16, name="cpack_b")
            nc.sync.dma_start(out=cpack_b, in_=cpack_b_d[:, :])
            cpack_f = singles.tile([128, _F32_W], f32, name="cpack_f")
            nc.sync.dma_start(out=cpack_f, in_=cpack_f_d[:, :])
            cpack_8 = singles.tile([128, _F8_W], f8, name="cpack_8")
            nc.sync.dma_start(out=cpack_8, in_=cpack_8_d[:, :])
            csb = {}
            for name, (r, c, off) in _BF16_LAYOUT.items():
                csb[name] = cpack_b[0:r, off:off + c]
            for name, (r, c, off) in _F32_LAYOUT.items():
                csb[name] = cpack_f[0:r, off:off + c]
            for name, (r, c, off) in _F8_LAYOUT.items():
                csb[name] = cpack_8[0:r, off:off + c]

            # persistent small PSUM banks; matmul PSUM writes must start at
            # partition 0/32/64.  wv (80 rows, consumed early each tile by
            # the e exp) rides the x3 pool rotation instead of owning a bank.
            SB1 = sbank_pool.tile([128, 512], f32, name="SB1")
            SB3a = sbank_pool.tile([128, 512], f32, name="SB3a")
            SB3b = sbank_pool.tile([128, 512], f32, name="SB3b")
            G_ps = SB1[0:16, 0:TN2]
            s8_ps = SB1[0:16, TN2:2 * TN2]
            # rsb/P double-buffered by tile parity: the tile's tail (stt
            # reads) is deferred past the NEXT tile's k-group matmuls
            rsb_bank = [SB3a[0:128, 0:TN2], SB3b[0:128, 0:TN2]]
            P_bank = [SB3a[:, TN2:2 * TN2], SB3b[:, TN2:2 * TN2]]

            KGROUPS = [(0, 2), (2, 2), (4, 2), (6, 2), (8, 2)]

            def dma_inputs(it):
                ft = bigio.tile([128, K * TN2], bf16, name="feats_t")
                nc.sync.dma_start(
                    out=ft, in_=feats_d[:, it * TN2 * K:(it + 1) * TN2 * K])
                p80 = p80pool.tile([80, 2 * TN2], bf16, name="pt80")
                nc.sync.dma_start(out=p80,
                                  in_=pt80_d[it].partition_broadcast(SP))
                return ft, p80

            def xn_chunk(xn_sb, feats_t, ci, dve=False):
                xn_ps = xnps_pool.tile([128, 512], f32, name="xn_ps",
                                       tag="xnps")
                mm(xn_ps, csb["w0ddT"], feats_t[:, ci * 512:(ci + 1) * 512],
                   start=True, stop=True)
                dst = xn_sb[:, 2 * ci:2 * ci + 2, :].rearrange(
                    "p a n -> p (a n)")
                if dve:
                    # prologue only: DVE is idle during pipeline fill
                    nc.vector.tensor_scalar(
                        out=dst, in0=xn_ps, scalar1=csb["b0dd"], scalar2=0.0,
                        op0=OP.add, op1=OP.max)
                else:
                    nc.scalar.activation(
                        out=dst, in_=xn_ps, func=AF.Relu, bias=csb["b0dd"])

            def gate_chain(it, xn_sb, pt80_sb, split=False):  # noqa: returns e, ws, m80
                """G -> h -> e logits -> exp -> HBM round trip launching the
                partition-replicated gate weights for tile `it`."""
                for j in range(K):
                    mm(G_ps, csb["dls"][:, 16 * j:16 * (j + 1)],
                       xn_sb[:, j, :], start=(j == 0), stop=False)
                mm(G_ps, csb["w1vA"], pt80_sb[0:K, 0:TN2], start=False,
                   stop=False)
                mm(G_ps, csb["w1vB"], pt80_sb[0:K, TN2:2 * TN2], start=False,
                   stop=True)
                h_sb = work.tile([16, TN2], bf16, name="h_sb")
                nc.scalar.activation(
                    out=h_sb, in_=G_ps, func=AF.Relu, bias=csb["cbdd"])
                e_sb = work.tile([80, 512], f8, name="e_sb")
                wv_t = xnps_pool.tile([128, 512], f32, name="wv_ps",
                                      tag="xnps")
                wv_ps = wv_t[0:80, :]
                mm(wv_ps[:, 0:TN2], csb["ww2A"], h_sb, start=True, stop=True)
                mm(wv_ps[:, TN2:2 * TN2], csb["ww2B"], h_sb, start=True,
                   stop=True)
                nc.scalar.activation(
                    out=e_sb, in_=wv_ps, func=AF.Exp, bias=csb["bw2col"])
                # per-channel gate weights = partition-replicated e via an
                # HBM round trip on the Pool SWDGE queue (engines cannot
                # replicate partitions; DMA can)
                ws_sb = work.tile([128, K, TN2], f8, name="ws_sb")
                sc = e_scr[it % 3]
                nc.gpsimd.dma_start(out=sc[0], in_=e_sb[:, 0:TN2])
                nc.sync.dma_start(out=sc[1], in_=e_sb[:, TN2:2 * TN2])
                for h, eng in ((0, nc.gpsimd), (1, nc.sync)):
                    wsrc = sc[h].rearrange("(g k) n -> g k n", g=SP) \
                        .partition_broadcast(SP)
                    if not split:
                        eng.dma_start(
                            out=ws_sb[64 * h:64 * h + 64, :, :], in_=wsrc)
                    else:
                        # pipeline fill only: early k-slices land first so
                        # tile 0's gate mults start ~1us sooner
                        eng.dma_start(
                            out=ws_sb[64 * h:64 * h + 64, 0:4, :],
                            in_=wsrc[:, :, 0:4, :])
                        eng.dma_start(
                            out=ws_sb[64 * h:64 * h + 64, 4:K, :],
                            in_=wsrc[:, :, 4:K, :])
                return e_sb, ws_sb

            # ---- prologue: 3 tiles of inputs, xn(0..1), gate chains ----
            feats_cur, pt80_cur = dma_inputs(0)
            feats_nxt, pt80_nxt = dma_inputs(1)
            feats_n2, pt80_n2 = dma_inputs(2)
            xn_cur = xnpool.tile([128, K, TN2], bf16, name="xn_sb")
            for ci in range(5):
                xn_chunk(xn_cur, feats_cur, ci, dve=(ci % 2 == 1))
            xn_nxt = xnpool.tile([128, K, TN2], bf16, name="xn_sb")
            for ci in range(5):
                xn_chunk(xn_nxt, feats_nxt, ci, dve=(ci % 2 == 1))
            e_cur, ws_cur = gate_chain(0, xn_cur, pt80_cur,
                                       split=True)
            e_nxt, ws_nxt = gate_chain(1, xn_nxt, pt80_nxt,
                                       split=True)

            pending_tail = None
            for it in range(NT):
                xn_sb, pt80_sb = xn_cur, pt80_cur
                e_sb, ws_sb = e_cur, ws_cur
                build = it + 2 < NT            # build tile it+2 this iter
                if it + 3 < NT:
                    feats_n3, pt80_n3 = dma_inputs(it + 3)
                if build:
                    xn_n2 = xnpool.tile([128, K, TN2], bf16, name="xn_sb")

                # gated-ptsn product in 80-row space; its k-sum happens
                # inside the vsel broadcast matmuls below
                m80_sb = work.tile([80, 2 * TN2], bf16, name="m80_sb")
                nc.vector.tensor_tensor(
                    out=m80_sb, in0=e_sb, in1=pt80_sb, op=OP.mult)

                # ---------- per-k-group: x3 matmul + gate product;
                # tile it+2's xn chunks fill the PE gaps ----------
                y_sb = work.tile([128, K, TN2], bf16, name="y_sb")
                rs8_sb = small.tile([16, TN2], f32, name="rs8_sb")
                for gi, (k0, kg) in enumerate(KGROUPS):
                    x3_ps = x3ps_pool.tile([128, 2, TN2], f32, name="x3_ps",
                                           tag="x3")
                    mm(x3_ps[:, 0:kg, :].rearrange("p a n -> p (a n)"),
                       csb["w3ddT"],
                       xn_sb[:, k0:k0 + kg, :].rearrange("p a n -> p (a n)"),
                       start=True, stop=True)
                    nc.vector.tensor_tensor(
                        out=y_sb[:, k0:k0 + kg, :],
                        in0=ws_sb[:, k0:k0 + kg, :], in1=x3_ps[:, 0:kg, :],
                        op=OP.mult)
                    if build:
                        xn_chunk(xn_n2, feats_n2, gi)
                    if gi == 0:
                        mm(s8_ps, csb["s8selA"], e_sb[:, 0:TN2], start=True,
                           stop=False)
                        mm(s8_ps, csb["s8selB"], e_sb[:, TN2:2 * TN2],
                           start=False, stop=True)
                        nc.vector.reciprocal_approx_fast(
                            out=rs8_sb, in_=s8_ps)
                        rs8b_sb = small.tile([16, TN2], bf16, name="rs8b_sb")
                        nc.gpsimd.tensor_copy(out=rs8b_sb, in_=rs8_sb)
                    elif gi == 1:
                        mm(rsb_bank[it % 2], csb["obcb"], rs8b_sb,
                           start=True, stop=True)
                    elif gi == 2:
                        mm(P_bank[it % 2], csb["vselA"], m80_sb[:, 0:TN2],
                           start=True, stop=False)
                        mm(P_bank[it % 2], csb["vselB"],
                           m80_sb[:, TN2:2 * TN2], start=False, stop=True)

                # ---------- weighted sum over k (tree) ----------
                nc.gpsimd.tensor_tensor(
                    out=y_sb[:, 0:5, :], in0=y_sb[:, 0:5, :],
                    in1=y_sb[:, 5:10, :], op=OP.add)
                nc.gpsimd.tensor_tensor(
                    out=y_sb[:, 0:2, :], in0=y_sb[:, 0:2, :],
                    in1=y_sb[:, 2:4, :], op=OP.add)
                t01_sb = small.tile([128, TN2], bf16, name="t01_sb")
                nc.gpsimd.tensor_tensor(
                    out=t01_sb, in0=y_sb[:, 0, :], in1=y_sb[:, 1, :],
                    op=OP.add)
                num_sb = small.tile([128, TN2], bf16, name="num_sb")
                nc.gpsimd.tensor_tensor(
                    out=num_sb, in0=t01_sb, in1=y_sb[:, 4, :], op=OP.add)

                # tail of the previous tile rides here, after this tile's
                # mults and tree are queued
                if pending_tail is not None:
                    pending_tail()
                    pending_tail = None

                def tail(num_sb=num_sb, xn_sb=xn_sb, it=it):
                    # normalize + relu + output head for tile `it`, deferred
                    # past tile it+1's k-group mults so the Pool-tree wait
                    # doesn't block DVE's queue between tiles.  (HW allows
                    # at most one PSUM input per DVE instruction, so the
                    # P-add and rsb-mult stay two separate stts.)
                    num2_sb = small.tile([128, TN2], bf16, name="num2_sb")
                    nc.vector.scalar_tensor_tensor(
                        out=num2_sb, in0=P_bank[it % 2], scalar=0.0,
                        in1=num_sb, op0=OP.bypass, op1=OP.add)
                    o1p_sb = small.tile([128, TN2], bf16, name="o1p_sb")
                    nc.vector.scalar_tensor_tensor(
                        out=o1p_sb, in0=num2_sb, scalar=0.0,
                        in1=rsb_bank[it % 2], op0=OP.bypass, op1=OP.mult)
                    o1_sb = small.tile([128, TN2], bf16, name="o1_sb")
                    nc.gpsimd.tensor_scalar(
                        out=o1_sb, in0=o1p_sb, scalar1=csb["b3dd"],
                        scalar2=0.0, op0=OP.add, op1=OP.max)
                    out2_ps = P_bank[it % 2]
                    mm(out2_ps, csb["woutddT"], o1_sb, start=True, stop=False)
                    mm(out2_ps, csb["idd"], xn_sb[:, 0, :], start=False,
                       stop=False)
                    mm(out2_ps, csb["boutrow"], csb["ones1"], start=False,
                       stop=True)
                    fin_sb = small.tile([128, TN2], bf16, name="fin_sb")
                    nc.scalar.copy(out=fin_sb, in_=out2_ps)
                    nc.sync.dma_start(
                        out=out_d[:, it * TN2:(it + 1) * TN2], in_=fin_sb)
                pending_tail = tail

                # ---------- gate chain two tiles ahead ----------
                if build:
                    e_n2, ws_n2 = gate_chain(it + 2, xn_n2, pt80_n2)
                    xn_cur, xn_nxt = xn_nxt, xn_n2
                    e_cur, ws_cur = e_nxt, ws_nxt
                    e_nxt, ws_nxt = e_n2, ws_n2
                    feats_cur, pt80_cur = feats_nxt, pt80_nxt
                    feats_nxt, pt80_nxt = feats_n2, pt80_n2
                    if it + 3 < NT:
                        feats_n2, pt80_n2 = feats_n3, pt80_n3
                elif it + 1 < NT:
                    xn_cur = xn_nxt
                    e_cur, ws_cur = e_nxt, ws_nxt
                    feats_cur, pt80_cur = feats_nxt, pt80_nxt

            pending_tail()

    nc.compile()
    return nc


def _fold_weights(inp):
    """Host-side weight folding -> dict of const arrays (f32)."""
    W0, b0 = inp["W0"], inp["b0"]
    W1, b1 = inp["W1"], inp["b1"]
    W2, b2 = inp["W2"], inp["b2"]
    W3, b3 = inp["W3"], inp["b3"]
    Wp1, Wp2 = inp["Wp1"], inp["Wp2"]
    Ww1, Ww2, bw2 = inp["Ww1"], inp["Ww2"], inp["bw2"]
    Wout, bout = inp["Wout"], inp["bout"]

    Ww1r = Ww1.reshape(GN, CH, K)
    A = Ww1r.sum(axis=2)
    AW1 = A @ W1
    C2 = np.einsum("omj,mc->ocj", Ww1r, W2)
    Dc = -C2.copy()
    Dc[:, :, 0] += AW1
    cb = A @ (b1 - b2)
    v = Wp2 @ np.maximum(Wp1[:, 0], 0.0)
    w1v = np.einsum("omj,m->oj", Ww1r, v)

    m64 = np.arange(CH)

    c = {}
    t = np.zeros((128, 128), np.float32)
    t[0:64, 0:64] = W0.T; t[64:128, 64:128] = W0.T
    c["w0ddT"] = t
    c["b0dd"] = np.concatenate([b0, b0]).reshape(128, 1)
    t = np.zeros((128, 128), np.float32)
    t[0:64, 0:64] = W3.T; t[64:128, 64:128] = W3.T
    c["w3ddT"] = t
    t = np.zeros((128, 10 * 16), np.float32)
    for j in range(K):
        t[0:64, 16 * j:16 * j + 8] = Dc[:, :, j].T
        t[64:128, 16 * j + 8:16 * j + 16] = Dc[:, :, j].T
    c["dls"] = t
    t = np.zeros((K, 16), np.float32)
    for j in range(K):
        t[j, 0:8] = w1v[:, j]
    c["w1vA"] = t
    t = np.zeros((K, 16), np.float32)
    for j in range(K):
        t[j, 8:16] = w1v[:, j]
    c["w1vB"] = t
    c["cbdd"] = np.concatenate([cb, cb]).reshape(16, 1).astype(np.float32)
    # vsel[g*10+k, c + 64h] = v[c] * [g == c % 8]: the per-k gated-ptsn
    # broadcast; contraction over the 80 rows sums over k for free.
    for h, nm in ((0, "vselA"), (1, "vselB")):
        t = np.zeros((80, 128), np.float32)
        for g in range(SP):
            for k in range(K):
                cc = m64[m64 % SP == g]
                t[g * K + k, cc + 64 * h] = v[cc]
        c[nm] = t
    t = np.zeros((16, 80), np.float32); t[0:8, :] = Ww2.T
    c["ww2A"] = t
    t = np.zeros((16, 80), np.float32); t[8:16, :] = Ww2.T
    c["ww2B"] = t
    c["bw2col"] = bw2.reshape(80, 1).astype(np.float32)
    t = np.zeros((80, 16), np.float32)
    for g in range(SP):
        for j in range(K):
            t[g * K + j, g] = 1.0
    c["s8selA"] = t
    t = np.zeros((80, 16), np.float32)
    for g in range(SP):
        for j in range(K):
            t[g * K + j, 8 + g] = 1.0
    c["s8selB"] = t
    t = np.zeros((16, 128), np.float32)
    for h in range(2):
        t[(m64 % SP) + 8 * h, m64 + 64 * h] = 1.0
    c["obcb"] = t
    t = np.zeros((128, 128), np.float32)
    t[0:64, 0:64] = Wout.T; t[64:128, 64:128] = Wout.T
    c["woutddT"] = t
    c["idd"] = np.eye(128, dtype=np.float32)
    c["b3dd"] = np.concatenate([b3, b3]).reshape(128, 1)
    c["boutdd"] = np.concatenate([bout, bout]).reshape(128, 1)
    c["boutrow"] = np.concatenate([bout, bout]).reshape(1, 128)
    c["ones1"] = np.ones((1, TN2), np.float32)
    return c


def make_in_maps(inputs):
    import ml_dtypes
    bf16 = ml_dtypes.bfloat16
    inp = {k: np.ascontiguousarray(np.asarray(v, dtype=np.float32))
           for k, v in inputs.items()}
    consts = _fold_weights(inp)
    cpack_b = np.zeros((128, _BF16_W), bf16)
    for name, (r, c, off) in _BF16_LAYOUT.items():
        cpack_b[0:r, off:off + c] = consts[name].astype(bf16)
    cpack_f = np.zeros((128, _F32_W), np.float32)
    for name, (r, c, off) in _F32_LAYOUT.items():
        cpack_f[0:r, off:off + c] = consts[name]
    f8 = ml_dtypes.float8_e4m3
    cpack_8 = np.zeros((128, _F8_W), f8)
    for name, (r, c, off) in _F8_LAYOUT.items():
        cpack_8[0:r, off:off + c] = consts[name].astype(f8)
    # host ptsn for all cores at once: [B, N, K]
    cent = inp["cent_pts"]                      # [B, N, 3]
    spt = inp["sm_pts"]                         # [B, 3, N, K]
    ptsn = ((cent.transpose(0, 2, 1)[:, :, :, None] - spt) ** 2).sum(axis=1)
    in_maps = []
    for b in range(B):
        m = {"cpack_b": cpack_b, "cpack_f": cpack_f,
             "cpack_8": cpack_8}
        # k-major pack: [64, NT, 2, TN2, K] -> [64, NT, 2, K, TN2]
        ff = inp["sm_feats"][b].reshape(CH, NT, 2, TN2, K)
        ff = ff.transpose(0, 1, 2, 4, 3)        # [64, NT, 2, K, TN2]
        m["feats"] = np.ascontiguousarray(
            np.concatenate([ff[:, :, 0], ff[:, :, 1]], axis=0)
            .reshape(128, N * K // 2).astype(bf16))
        # pt80[it, k, h*256+n] = ptsn[it, h, n, k]; replicated on-device
        m["pt80"] = np.ascontiguousarray(
            ptsn[b].reshape(NT, 2, TN2, K).transpose(0, 3, 1, 2)
            .reshape(NT, K, 2 * TN2).astype(bf16))
        in_maps.append(m)
    return in_maps


def _run(inputs, trace=False):
    from concourse.bass_utils import run_bass_kernel_spmd

    if "nc" not in _CACHE:
        _CACHE["nc"] = _build_bass()
    nc = _CACHE["nc"]
    in_maps = make_in_maps(inputs)

    res = run_bass_kernel_spmd(
        nc, in_maps, core_ids=list(range(B)), trace=trace)
    outs = []
    for r in res.results:
        o = np.asarray(r["out"]).astype(np.float32) \
            .reshape(2, CH, NT, TN2)               # [half, c, tile, n]
        outs.append(np.ascontiguousarray(
            o.transpose(1, 2, 0, 3).reshape(CH, N)))
    out = np.stack(outs, axis=0)
    return out, res


def kernel(**inputs) -> np.ndarray:
    out, _ = _run(inputs, trace=False)
    return out



# revision 20
# speedup vs baseline: 1.5445x; 1.5445x over previous
"""Point-Transformer block as a Bass/Tile kernel for 8 Trainium2 NeuronCores.

Strategy
--------
Data-parallel over batch: core b handles batch element b (B == 8 == n_cores).

Host-side algebraic folding (all on 64x64-ish weights, negligible cost):
  * x1/x2 are never materialized: the gate-logit path folds into per-j
    64->8 matmuls with weights D_j = -Ww1_j@W2 (+ Ww1-rowsum@W1 for j==0).
  * pt_conv collapses: ptsn >= 0 so relu(Wp1*ptsn) = relu(Wp1)*ptsn, hence
    ptf = v (x) ptsn with v = Wp2 @ relu(Wp1); its contribution to the
    gated sum is computed in 80-row (group,k) space and folded into two
    `vsel` broadcast matmuls whose contraction performs the k-sum.
  * b3 is deferred through the softmax (sum_k ws = 1); bout rides a
    ones-row matmul inside the output-head PSUM accumulation.

Layout: 512-point tiles, the two 256-point halves packed on partitions
[0:64)/[64:128) so all 64-channel matmuls run with block-diagonal weights
at full PE occupancy.  feats are bf16 and k-major in HBM so xn PSUM
chunks are exactly [128, 512]; activations/weights are bf16 (matmul cols
then cost 1 PE cycle); the gate weights e are fp8e4 (the softmax
normalization cancels most of the quantization).

The per-channel gate weights ws[c,k,n] = e[(c%8)*10+k,n] are a pure
partition replication of e: engines cannot replicate partitions, but PE
selector matmuls can -- per k, two [80,64] 0/1 matmuls place the two
point-halves on PSUM partitions 0:64/64:128, one Act copy per k-pair
lands f8 ws_sb.  (An earlier HBM-round-trip gather relied on per-queue
DMA FIFO ordering that HWDGE does not guarantee across transfer shapes
and raced intermittently; PE replication is fully dependency-tracked.)

Software pipeline (per iteration): consume tile it (x3 matmuls + DVE
gate products + Pool k-reduction tree), build tile it+2's xn and launch
its gate chain (G -> h -> e -> exp -> HBM round trip) so the ~9us chain
latency hides under two tile periods; the normalize/relu/head tail of
tile it-1 is emitted after tile it's mults so its Pool-tree wait never
head-blocks DVE's queue; rsb/P PSUM banks are parity double-buffered.
GPSIMD never touches PSUM (illegal on HW); matmul PSUM writes start at
partition 0/32/64 only; DVE instructions read at most one PSUM operand.

I/O: ptsn ships unreplicated and is expanded on-device by a
partition-broadcast load DMA; the output is written bf16 (host upcasts
after the gather).  The prologue xn relus alternate Act/DVE (DVE is
idle until the first gate weights arrive).

Dispatch-path packing: the per-dispatch cost of this environment is
dominated by per-tensor and per-byte I/O overhead, not device compute,
so ALL inputs ship as ONE 1-D int8 blob per core (feats quantized to
int8 with an MSE-optimal clip, scale folded into w0ddT on the host;
pt80/consts ride as raw bitcast bytes).  feats are converted int8->bf16
on device (split across Pool and Act) before the xn matmuls.
"""

import numpy as np

B, N, K = 8, 8192, 10
CH = 64          # IN == MID == OUT
SP = 8
GN = CH // SP    # 8 gate channels
TN = 512         # points per tile
TN2 = TN // 2    # points per partition-half
NT = N // TN     # 16 tiles

# packed bf16 const layout: name -> (rows, cols, col offset)
_BF16_LAYOUT = {}
_off = 0
for _name, _r, _c in [
    ("w0ddT", 128, 128), ("w3ddT", 128, 128), ("dls", 128, 160),
    ("w1vA", 10, 16), ("w1vB", 10, 16),
    ("ww2A", 16, 80), ("ww2B", 16, 80), ("s8selA", 80, 16),
    ("vselA", 80, 128), ("vselB", 80, 128),
    ("woutddT", 128, 128), ("idd", 128, 128), ("boutrow", 1, 128),
    ("ones1", 1, 256), ("obcb", 16, 128),
]:
    _BF16_LAYOUT[_name] = (_r, _c, _off)
    _off += _c
_BF16_W = _off

_F8_LAYOUT = {}
_off = 0
for _name, _r, _c in [
    ("s8selA", 80, 16), ("s8selB", 80, 16), ("wrep", 80, 640),
]:
    _F8_LAYOUT[_name] = (_r, _c, _off)
    _off += _c
_F8_W = _off

_F32_LAYOUT = {}
_off = 0
for _name, _r, _c in [
    ("b0dd", 128, 1), ("cbdd", 16, 1), ("b3dd", 128, 1),
    ("bw2col", 80, 1), ("boutdd", 128, 1),
]:
    _F32_LAYOUT[_name] = (_r, _c, _off)
    _off += _c
_F32_W = _off

# single-blob byte layout (all regions 512B-aligned)
OFF_FEATS = 0
FEATS_BYTES = 128 * (N * K // 2)             # int8, k-major packed
OFF_PT80 = OFF_FEATS + FEATS_BYTES
PT80_BYTES = NT * K * 2 * TN2 * 2            # bf16 [NT, K, 2*TN2]
OFF_CB = OFF_PT80 + PT80_BYTES
CB_BYTES = 128 * _BF16_W * 2
OFF_CF = OFF_CB + CB_BYTES
CF_BYTES = 128 * _F32_W * 4
OFF_C8 = OFF_CF + CF_BYTES
C8_BYTES = 128 * _F8_W
TOTAL_BYTES = OFF_C8 + C8_BYTES
assert all(o % 512 == 0 for o in (OFF_PT80, OFF_CB, OFF_CF, OFF_C8))

_CACHE = {}


def _build_bass():
    import concourse.bacc as bacc
    import concourse.tile as tile
    from concourse import mybir

    f32 = mybir.dt.float32
    f32r = mybir.dt.float32r
    bf16 = mybir.dt.bfloat16
    f8 = mybir.dt.float8e4
    i8 = mybir.dt.int8
    AF = mybir.ActivationFunctionType
    OP = mybir.AluOpType

    nc = bacc.Bacc("TRN2", target_bir_lowering=False)

    def mm(out, lhsT, rhs, **kw):
        nc.tensor.matmul(out, lhsT, rhs, **kw)

    # ---------------- DRAM I/O ----------------
    # ONE packed input blob per core: the dispatch path charges ~1.5ms
    # per external tensor per iteration plus a per-byte toll, so all
    # inputs ride in a single 1-D int8 tensor and are bitcast on access.
    blob_d = nc.dram_tensor("blob", [TOTAL_BYTES], i8,
                            kind="ExternalInput")
    # feats pre-packed on host: [c + 64*half, ((tile, k), within)] k-major,
    # quantized int8 (scale folded into w0ddT host-side)
    feats_v = blob_d[OFF_FEATS:OFF_FEATS + FEATS_BYTES] \
        .rearrange("(p x) -> p x", p=128)
    # ptsn per tile, [tile, k, h*256+n]; the 8-group replication happens
    # in the load DMA via a partition-broadcast access pattern
    pt80_v = blob_d[OFF_PT80:OFF_PT80 + PT80_BYTES].bitcast(bf16) \
        .rearrange("(t k n) -> t k n", t=NT, k=K)
    cpack_b_v = blob_d[OFF_CB:OFF_CB + CB_BYTES].bitcast(bf16) \
        .rearrange("(p x) -> p x", p=128)
    cpack_f_v = blob_d[OFF_CF:OFF_CF + CF_BYTES].bitcast(f32) \
        .rearrange("(p x) -> p x", p=128)
    cpack_8_v = blob_d[OFF_C8:OFF_C8 + C8_BYTES].bitcast(f8) \
        .rearrange("(p x) -> p x", p=128)
    out_d = nc.dram_tensor("out", [128, N // 2], bf16,
                           kind="ExternalOutput")

    with tile.TileContext(nc) as tc:
        with (
            tc.tile_pool(name="singles", bufs=1) as singles,
            tc.tile_pool(name="bigio", bufs=3) as bigio,
            tc.tile_pool(name="fconv", bufs=2) as fconv,
            tc.tile_pool(name="xnpool", bufs=4) as xnpool,
            tc.tile_pool(name="p80pool", bufs=4) as p80pool,
            tc.tile_pool(name="work", bufs=3) as work,
            tc.tile_pool(name="small", bufs=3) as small,
            tc.tile_pool(name="xnps", bufs=2, space="PSUM") as xnps_pool,
            tc.tile_pool(name="x3ps", bufs=3, space="PSUM") as x3ps_pool,
            tc.tile_pool(name="sbank", bufs=1, space="PSUM") as sbank_pool,
        ):
            # ---- packed consts in SBUF (three DMAs from the blob) ----
            cpack_b = singles.tile([128, _BF16_W], bf16, name="cpack_b")
            nc.sync.dma_start(out=cpack_b, in_=cpack_b_v[:, :])
            cpack_f = singles.tile([128, _F32_W], f32, name="cpack_f")
            nc.sync.dma_start(out=cpack_f, in_=cpack_f_v[:, :])
            cpack_8 = singles.tile([128, _F8_W], f8, name="cpack_8")
            nc.sync.dma_start(out=cpack_8, in_=cpack_8_v[:, :])
            csb = {}
            for name, (r, c, off) in _BF16_LAYOUT.items():
                csb[name] = cpack_b[0:r, off:off + c]
            for name, (r, c, off) in _F32_LAYOUT.items():
                csb[name] = cpack_f[0:r, off:off + c]
            for name, (r, c, off) in _F8_LAYOUT.items():
                csb[name] = cpack_8[0:r, off:off + c]

            # persistent small PSUM banks; matmul PSUM writes must start at
            # partition 0/32/64.  wv (80 rows, consumed early each tile by
            # the e exp) rides the x3 pool rotation instead of owning a bank.
            SB1 = sbank_pool.tile([128, 512], f32, name="SB1")
            SB3a = sbank_pool.tile([128, 512], f32, name="SB3a")
            SB3b = sbank_pool.tile([128, 512], f32, name="SB3b")
            G_ps = SB1[0:16, 0:TN2]
            s8_ps = SB1[0:16, TN2:2 * TN2]
            # rsb/P double-buffered by tile parity: the tile's tail (stt
            # reads) is deferred past the NEXT tile's k-group matmuls
            rsb_bank = [SB3a[0:128, 0:TN2], SB3b[0:128, 0:TN2]]
            P_bank = [SB3a[:, TN2:2 * TN2], SB3b[:, TN2:2 * TN2]]

            KGROUPS = [(0, 2), (2, 2), (4, 2), (6, 2), (8, 2)]

            def dma_inputs(it):
                ft8 = bigio.tile([128, K * TN2], i8, name="feats8_t")
                nc.sync.dma_start(
                    out=ft8, in_=feats_v[:, it * TN2 * K:(it + 1) * TN2 * K])
                p80 = p80pool.tile([80, 2 * TN2], bf16, name="pt80")
                nc.sync.dma_start(out=p80,
                                  in_=pt80_v[it].partition_broadcast(SP))
                return ft8, p80

            def conv_feats(ft8):
                # int8 -> bf16 (exact) for the PE; split Pool/Act so
                # neither engine eats the whole 2560 el/partition
                ftb = fconv.tile([128, K * TN2], bf16, name="featsb_t")
                hw = K * TN2 // 2
                nc.gpsimd.tensor_copy(out=ftb[:, 0:hw], in_=ft8[:, 0:hw])
                nc.scalar.copy(out=ftb[:, hw:], in_=ft8[:, hw:])
                return ftb

            def xn_chunk(xn_sb, feats_t, ci, dve=False):
                xn_ps = xnps_pool.tile([128, 512], f32, name="xn_ps",
                                       tag="xnps")
                mm(xn_ps, csb["w0ddT"], feats_t[:, ci * 512:(ci + 1) * 512],
                   start=True, stop=True)
                dst = xn_sb[:, 2 * ci:2 * ci + 2, :].rearrange(
                    "p a n -> p (a n)")
                if dve:
                    # prologue only: DVE is idle during pipeline fill
                    nc.vector.tensor_scalar(
                        out=dst, in0=xn_ps, scalar1=csb["b0dd"], scalar2=0.0,
                        op0=OP.add, op1=OP.max)
                else:
                    nc.scalar.activation(
                        out=dst, in_=xn_ps, func=AF.Relu, bias=csb["b0dd"])

            def gate_chain(it, xn_sb, pt80_sb):  # noqa: returns e, ws
                """G -> h -> e logits -> exp -> PE selector replication of
                the per-channel gate weights for tile `it`."""
                for j in range(K):
                    mm(G_ps, csb["dls"][:, 16 * j:16 * (j + 1)],
                       xn_sb[:, j, :], start=(j == 0), stop=False)
                mm(G_ps, csb["w1vA"], pt80_sb[0:K, 0:TN2], start=False,
                   stop=False)
                mm(G_ps, csb["w1vB"], pt80_sb[0:K, TN2:2 * TN2], start=False,
                   stop=True)
                h_sb = work.tile([16, TN2], bf16, name="h_sb")
                nc.scalar.activation(
                    out=h_sb, in_=G_ps, func=AF.Relu, bias=csb["cbdd"])
                e_sb = work.tile([80, 512], f8, name="e_sb")
                wv_t = xnps_pool.tile([128, 512], f32, name="wv_ps",
                                      tag="xnps")
                wv_ps = wv_t[0:80, :]
                mm(wv_ps[:, 0:TN2], csb["ww2A"], h_sb, start=True, stop=True)
                mm(wv_ps[:, TN2:2 * TN2], csb["ww2B"], h_sb, start=True,
                   stop=True)
                nc.scalar.activation(
                    out=e_sb, in_=wv_ps, func=AF.Exp, bias=csb["bw2col"])
                # per-channel gate weights ws[c+64h, k, n] = e[(c%8)*10+k,
                # h*256+n]: a pure partition replication of e.  Engines
                # cannot replicate partitions but PE selector matmuls can:
                # per k, two [80, 64] 0/1-stationary matmuls place the two
                # point-halves on partitions 0:64 / 64:128 of one PSUM
                # half-bank, then one Act copy per k-pair lands f8 ws_sb.
                # (All deps SBUF/PSUM-tracked -- no HBM round trip, whose
                # write->gather ordering relied on per-queue DMA FIFO that
                # HWDGE does not guarantee across transfer shapes.)
                ws_sb = work.tile([128, K, TN2], f8, name="ws_sb")
                for kp in range(K // 2):
                    wr = xnps_pool.tile([128, 512], f32, name="wsrep_ps",
                                        tag="xnps")
                    for dk in range(2):
                        k = 2 * kp + dk
                        rk = csb["wrep"][:, 64 * k:64 * k + 64]
                        mm(wr[0:64, dk * TN2:(dk + 1) * TN2], rk,
                           e_sb[:, 0:TN2], start=True, stop=True)
                        mm(wr[64:128, dk * TN2:(dk + 1) * TN2], rk,
                           e_sb[:, TN2:2 * TN2], start=True, stop=True)
                    nc.scalar.copy(
                        out=ws_sb[:, 2 * kp:2 * kp + 2, :].rearrange(
                            "p a n -> p (a n)"), in_=wr)
                return e_sb, ws_sb

            # ---- prologue: 3 tiles of inputs, xn(0..1), gate chains ----
            feats_cur, pt80_cur = dma_inputs(0)
            feats_nxt, pt80_nxt = dma_inputs(1)
            feats_n2, pt80_n2 = dma_inputs(2)
            xn_cur = xnpool.tile([128, K, TN2], bf16, name="xn_sb")
            ftb = conv_feats(feats_cur)
            for ci in range(5):
                xn_chunk(xn_cur, ftb, ci, dve=(ci % 2 == 1))
            xn_nxt = xnpool.tile([128, K, TN2], bf16, name="xn_sb")
            ftb = conv_feats(feats_nxt)
            for ci in range(5):
                xn_chunk(xn_nxt, ftb, ci, dve=(ci % 2 == 1))
            e_cur, ws_cur = gate_chain(0, xn_cur, pt80_cur)
            e_nxt, ws_nxt = gate_chain(1, xn_nxt, pt80_nxt)

            pending_tail = None
            for it in range(NT):
                xn_sb, pt80_sb = xn_cur, pt80_cur
                e_sb, ws_sb = e_cur, ws_cur
                build = it + 2 < NT            # build tile it+2 this iter
                if it + 3 < NT:
                    feats_n3, pt80_n3 = dma_inputs(it + 3)
                if build:
                    xn_n2 = xnpool.tile([128, K, TN2], bf16, name="xn_sb")
                    ftb_n2 = conv_feats(feats_n2)

                # gated-ptsn product in 80-row space; its k-sum happens
                # inside the vsel broadcast matmuls below
                m80_sb = work.tile([80, 2 * TN2], bf16, name="m80_sb")
                nc.vector.tensor_tensor(
                    out=m80_sb, in0=e_sb, in1=pt80_sb, op=OP.mult)

                # ---------- per-k-group: x3 matmul + gate product;
                # tile it+2's xn chunks fill the PE gaps ----------
                y_sb = work.tile([128, K, TN2], bf16, name="y_sb")
                rs8_sb = small.tile([16, TN2], f32, name="rs8_sb")
                for gi, (k0, kg) in enumerate(KGROUPS):
                    x3_ps = x3ps_pool.tile([128, 2, TN2], f32, name="x3_ps",
                                           tag="x3")
                    mm(x3_ps[:, 0:kg, :].rearrange("p a n -> p (a n)"),
                       csb["w3ddT"],
                       xn_sb[:, k0:k0 + kg, :].rearrange("p a n -> p (a n)"),
                       start=True, stop=True)
                    nc.vector.tensor_tensor(
                        out=y_sb[:, k0:k0 + kg, :],
                        in0=ws_sb[:, k0:k0 + kg, :], in1=x3_ps[:, 0:kg, :],
                        op=OP.mult)
                    if build:
                        xn_chunk(xn_n2, ftb_n2, gi)
                    if gi == 0:
                        mm(s8_ps, csb["s8selA"], e_sb[:, 0:TN2], start=True,
                           stop=False)
                        mm(s8_ps, csb["s8selB"], e_sb[:, TN2:2 * TN2],
                           start=False, stop=True)
                        nc.vector.reciprocal_approx_fast(
                            out=rs8_sb, in_=s8_ps)
                        rs8b_sb = small.tile([16, TN2], bf16, name="rs8b_sb")
                        nc.gpsimd.tensor_copy(out=rs8b_sb, in_=rs8_sb)
                    elif gi == 1:
                        mm(rsb_bank[it % 2], csb["obcb"], rs8b_sb,
                           start=True, stop=True)
                    elif gi == 2:
                        mm(P_bank[it % 2], csb["vselA"], m80_sb[:, 0:TN2],
                           start=True, stop=False)
                        mm(P_bank[it % 2], csb["vselB"],
                           m80_sb[:, TN2:2 * TN2], start=False, stop=True)

                # ---------- weighted sum over k (tree) ----------
                nc.gpsimd.tensor_tensor(
                    out=y_sb[:, 0:5, :], in0=y_sb[:, 0:5, :],
                    in1=y_sb[:, 5:10, :], op=OP.add)
                nc.gpsimd.tensor_tensor(
                    out=y_sb[:, 0:2, :], in0=y_sb[:, 0:2, :],
                    in1=y_sb[:, 2:4, :], op=OP.add)
                t01_sb = small.tile([128, TN2], bf16, name="t01_sb")
                nc.gpsimd.tensor_tensor(
                    out=t01_sb, in0=y_sb[:, 0, :], in1=y_sb[:, 1, :],
                    op=OP.add)
                num_sb = small.tile([128, TN2], bf16, name="num_sb")
                nc.gpsimd.tensor_tensor(
                    out=num_sb, in0=t01_sb, in1=y_sb[:, 4, :], op=OP.add)

                # tail of the previous tile rides here, after this tile's
                # mults and tree are queued
                if pending_tail is not None:
                    pending_tail()
                    pending_tail = None

                def tail(num_sb=num_sb, xn_sb=xn_sb, it=it):
                    # normalize + relu + output head for tile `it`, deferred
                    # past tile it+1's k-group mults so the Pool-tree wait
                    # doesn't block DVE's queue between tiles.  (HW allows
                    # at most one PSUM input per DVE instruction, so the
                    # P-add and rsb-mult stay two separate stts.)
                    num2_sb = small.tile([128, TN2], bf16, name="num2_sb")
                    nc.vector.scalar_tensor_tensor(
                        out=num2_sb, in0=P_bank[it % 2], scalar=0.0,
                        in1=num_sb, op0=OP.bypass, op1=OP.add)
                    o1p_sb = small.tile([128, TN2], bf16, name="o1p_sb")
                    nc.vector.scalar_tensor_tensor(
                        out=o1p_sb, in0=num2_sb, scalar=0.0,
                        in1=rsb_bank[it % 2], op0=OP.bypass, op1=OP.mult)
                    o1_sb = small.tile([128, TN2], bf16, name="o1_sb")
                    nc.gpsimd.tensor_scalar(
                        out=o1_sb, in0=o1p_sb, scalar1=csb["b3dd"],
                        scalar2=0.0, op0=OP.add, op1=OP.max)
                    out2_ps = P_bank[it % 2]
                    mm(out2_ps, csb["woutddT"], o1_sb, start=True, stop=False)
                    mm(out2_ps, csb["idd"], xn_sb[:, 0, :], start=False,
                       stop=False)
                    mm(out2_ps, csb["boutrow"], csb["ones1"], start=False,
                       stop=True)
                    fin_sb = small.tile([128, TN2], bf16, name="fin_sb")
                    nc.scalar.copy(out=fin_sb, in_=out2_ps)
                    nc.sync.dma_start(
                        out=out_d[:, it * TN2:(it + 1) * TN2], in_=fin_sb)
                pending_tail = tail

                # ---------- gate chain two tiles ahead ----------
                if build:
                    e_n2, ws_n2 = gate_chain(it + 2, xn_n2, pt80_n2)
                    xn_cur, xn_nxt = xn_nxt, xn_n2
                    e_cur, ws_cur = e_nxt, ws_nxt
                    e_nxt, ws_nxt = e_n2, ws_n2
                    feats_cur, pt80_cur = feats_nxt, pt80_nxt
                    feats_nxt, pt80_nxt = feats_n2, pt80_n2
                    if it + 3 < NT:
                        feats_n2, pt80_n2 = feats_n3, pt80_n3
                elif it + 1 < NT:
                    xn_cur = xn_nxt
                    e_cur, ws_cur = e_nxt, ws_nxt
                    feats_cur, pt80_cur = feats_nxt, pt80_nxt

            pending_tail()

    nc.compile()
    return nc


def _fold_weights(inp):
    """Host-side weight folding -> dict of const arrays (f32)."""
    W0, b0 = inp["W0"], inp["b0"]
    W1, b1 = inp["W1"], inp["b1"]
    W2, b2 = inp["W2"], inp["b2"]
    W3, b3 = inp["W3"], inp["b3"]
    Wp1, Wp2 = inp["Wp1"], inp["Wp2"]
    Ww1, Ww2, bw2 = inp["Ww1"], inp["Ww2"], inp["bw2"]
    Wout, bout = inp["Wout"], inp["bout"]

    Ww1r = Ww1.reshape(GN, CH, K)
    A = Ww1r.sum(axis=2)
    AW1 = A @ W1
    C2 = np.einsum("omj,mc->ocj", Ww1r, W2)
    Dc = -C2.copy()
    Dc[:, :, 0] += AW1
    cb = A @ (b1 - b2)
    v = Wp2 @ np.maximum(Wp1[:, 0], 0.0)
    w1v = np.einsum("omj,m->oj", Ww1r, v)

    m64 = np.arange(CH)

    c = {}
    t = np.zeros((128, 128), np.float32)
    t[0:64, 0:64] = W0.T; t[64:128, 64:128] = W0.T
    c["w0ddT"] = t
    c["b0dd"] = np.concatenate([b0, b0]).reshape(128, 1)
    t = np.zeros((128, 128), np.float32)
    t[0:64, 0:64] = W3.T; t[64:128, 64:128] = W3.T
    c["w3ddT"] = t
    t = np.zeros((128, 10 * 16), np.float32)
    for j in range(K):
        t[0:64, 16 * j:16 * j + 8] = Dc[:, :, j].T
        t[64:128, 16 * j + 8:16 * j + 16] = Dc[:, :, j].T
    c["dls"] = t
    t = np.zeros((K, 16), np.float32)
    for j in range(K):
        t[j, 0:8] = w1v[:, j]
    c["w1vA"] = t
    t = np.zeros((K, 16), np.float32)
    for j in range(K):
        t[j, 8:16] = w1v[:, j]
    c["w1vB"] = t
    c["cbdd"] = np.concatenate([cb, cb]).reshape(16, 1).astype(np.float32)
    # vsel[g*10+k, c + 64h] = v[c] * [g == c % 8]: the per-k gated-ptsn
    # broadcast; contraction over the 80 rows sums over k for free.
    for h, nm in ((0, "vselA"), (1, "vselB")):
        t = np.zeros((80, 128), np.float32)
        for g in range(SP):
            for k in range(K):
                cc = m64[m64 % SP == g]
                t[g * K + k, cc + 64 * h] = v[cc]
        c[nm] = t
    t = np.zeros((16, 80), np.float32); t[0:8, :] = Ww2.T
    c["ww2A"] = t
    t = np.zeros((16, 80), np.float32); t[8:16, :] = Ww2.T
    c["ww2B"] = t
    c["bw2col"] = bw2.reshape(80, 1).astype(np.float32)
    t = np.zeros((80, 16), np.float32)
    for g in range(SP):
        for j in range(K):
            t[g * K + j, g] = 1.0
    c["s8selA"] = t
    t = np.zeros((80, 16), np.float32)
    for g in range(SP):
        for j in range(K):
            t[g * K + j, 8 + g] = 1.0
    c["s8selB"] = t
    # ws replication selectors: block k is [80, 64] with
    # wrep[(c%8)*10 + k, c] = 1 (shared by both point-halves)
    t = np.zeros((80, 640), np.float32)
    for k in range(K):
        for cc in range(64):
            t[(cc % SP) * K + k, 64 * k + cc] = 1.0
    c["wrep"] = t
    t = np.zeros((16, 128), np.float32)
    for h in range(2):
        t[(m64 % SP) + 8 * h, m64 + 64 * h] = 1.0
    c["obcb"] = t
    t = np.zeros((128, 128), np.float32)
    t[0:64, 0:64] = Wout.T; t[64:128, 64:128] = Wout.T
    c["woutddT"] = t
    c["idd"] = np.eye(128, dtype=np.float32)
    c["b3dd"] = np.concatenate([b3, b3]).reshape(128, 1)
    c["boutdd"] = np.concatenate([bout, bout]).reshape(128, 1)
    c["boutrow"] = np.concatenate([bout, bout]).reshape(1, 128)
    c["ones1"] = np.ones((1, TN2), np.float32)
    return c


def make_in_maps(inputs):
    import ml_dtypes
    bf16 = ml_dtypes.bfloat16
    inp = {k: np.ascontiguousarray(np.asarray(v, dtype=np.float32))
           for k, v in inputs.items()}
    consts = _fold_weights(inp)
    cpack_f = np.zeros((128, _F32_W), np.float32)
    for name, (r, c, off) in _F32_LAYOUT.items():
        cpack_f[0:r, off:off + c] = consts[name]
    f8 = ml_dtypes.float8_e4m3
    cpack_8 = np.zeros((128, _F8_W), f8)
    for name, (r, c, off) in _F8_LAYOUT.items():
        cpack_8[0:r, off:off + c] = consts[name].astype(f8)
    cf_bytes = np.frombuffer(cpack_f.tobytes(), np.int8)
    c8_bytes = np.frombuffer(cpack_8.tobytes(), np.int8)
    # host ptsn for all cores at once: [B, N, K]
    cent = inp["cent_pts"]                      # [B, N, 3]
    spt = inp["sm_pts"]                         # [B, 3, N, K]
    ptsn = ((cent.transpose(0, 2, 1)[:, :, :, None] - spt) ** 2).sum(axis=1)
    in_maps = []
    for b in range(B):
        # k-major pack: [64, NT, 2, TN2, K] -> [64, NT, 2, K, TN2]
        ff = inp["sm_feats"][b].reshape(CH, NT, 2, TN2, K)
        ff = ff.transpose(0, 1, 2, 4, 3)        # [64, NT, 2, K, TN2]
        fpk = np.ascontiguousarray(
            np.concatenate([ff[:, :, 0], ff[:, :, 1]], axis=0)
            .reshape(128, N * K // 2))
        # int8 quantization, MSE-optimal clip (coarse subsampled scan);
        # the scale folds into this core's w0ddT below
        amax = float(np.abs(fpk).max())
        sub = fpk.reshape(-1)[::17]
        best_s, best_mse = None, np.inf
        for frac in (0.68, 0.71, 0.74, 0.77, 0.80, 1.0):
            s = frac * amax / 127.0
            qs = np.clip(np.round(sub / s), -127, 127)
            mse = float(((qs * s - sub) ** 2).mean())
            if mse < best_mse:
                best_s, best_mse = s, mse
        fq = np.clip(np.round(fpk / best_s), -127, 127).astype(np.int8)
        cpack_b = np.zeros((128, _BF16_W), bf16)
        for name, (r, c, off) in _BF16_LAYOUT.items():
            arr = consts[name]
            if name == "w0ddT":
                arr = arr * best_s
            cpack_b[0:r, off:off + c] = arr.astype(bf16)
        # pt80[it, k, h*256+n] = ptsn[it, h, n, k]; replicated on-device
        pt80 = np.ascontiguousarray(
            ptsn[b].reshape(NT, 2, TN2, K).transpose(0, 3, 1, 2)
            .reshape(NT, K, 2 * TN2).astype(bf16))
        blob = np.zeros(TOTAL_BYTES, np.int8)
        blob[OFF_FEATS:OFF_FEATS + FEATS_BYTES] = fq.reshape(-1)
        blob[OFF_PT80:OFF_PT80 + PT80_BYTES] = \
            np.frombuffer(pt80.tobytes(), np.int8)
        blob[OFF_CB:OFF_CB + CB_BYTES] = \
            np.frombuffer(cpack_b.tobytes(), np.int8)
        blob[OFF_CF:OFF_CF + CF_BYTES] = cf_bytes
        blob[OFF_C8:OFF_C8 + C8_BYTES] = c8_bytes
        in_maps.append({"blob": blob})
    return in_maps


def _run(inputs, trace=False):
    from concourse.bass_utils import run_bass_kernel_spmd

    if "nc" not in _CACHE:
        _CACHE["nc"] = _build_bass()
    nc = _CACHE["nc"]
    in_maps = make_in_maps(inputs)

    res = run_bass_kernel_spmd(
        nc, in_maps, core_ids=list(range(B)), trace=trace)
    outs = []
    for r in res.results:
        o = np.asarray(r["out"]).astype(np.float32) \
            .reshape(2, CH, NT, TN2)               # [half, c, tile, n]
        outs.append(np.ascontiguousarray(
            o.transpose(1, 2, 0, 3).reshape(CH, N)))
    out = np.stack(outs, axis=0)
    return out, res


def kernel(**inputs) -> np.ndarray:
    out, _ = _run(inputs, trace=False)
    return out



# revision 21
# speedup vs baseline: 1.6305x; 1.0557x over previous
"""Point-Transformer block as a Bass/Tile kernel for 8 Trainium2 NeuronCores.

Strategy
--------
Data-parallel over batch: core b handles batch element b (B == 8 == n_cores).

Host-side algebraic folding (all on 64x64-ish weights, negligible cost):
  * x1/x2 are never materialized: the gate-logit path folds into per-j
    64->8 matmuls with weights D_j = -Ww1_j@W2 (+ Ww1-rowsum@W1 for j==0).
  * pt_conv collapses: ptsn >= 0 so relu(Wp1*ptsn) = relu(Wp1)*ptsn, hence
    ptf = v (x) ptsn with v = Wp2 @ relu(Wp1); its contribution to the
    gated sum is computed in 80-row (group,k) space and folded into two
    `vsel` broadcast matmuls whose contraction performs the k-sum.
  * b3 is deferred through the softmax (sum_k ws = 1); bout rides a
    ones-row matmul inside the output-head PSUM accumulation.

Layout: 512-point tiles, the two 256-point halves packed on partitions
[0:64)/[64:128) so all 64-channel matmuls run with block-diagonal weights
at full PE occupancy.  feats are bf16 and k-major in HBM so xn PSUM
chunks are exactly [128, 512]; activations/weights are bf16 (matmul cols
then cost 1 PE cycle); the gate weights e are fp8e4 (the softmax
normalization cancels most of the quantization).

The per-channel gate weights ws[c,k,n] = e[(c%8)*10+k,n] are a pure
partition replication of e: engines cannot replicate partitions, but PE
selector matmuls can -- per k, two [80,64] 0/1 matmuls place the two
point-halves on PSUM partitions 0:64/64:128, one Act copy per k-pair
lands f8 ws_sb.  (An earlier HBM-round-trip gather relied on per-queue
DMA FIFO ordering that HWDGE does not guarantee across transfer shapes
and raced intermittently; PE replication is fully dependency-tracked.)

Software pipeline (per iteration): consume tile it (x3 matmuls + DVE
gate products + Pool k-reduction tree), build tile it+2's xn and launch
its gate chain (G -> h -> e -> exp -> PE ws replication) so the chain
latency hides under two tile periods; the normalize/relu/head tail of
tile it-1 is emitted after tile it's mults so its Pool-tree wait never
head-blocks DVE's queue; rsb/P PSUM banks are parity double-buffered.
GPSIMD never touches PSUM (illegal on HW); matmul PSUM writes start at
partition 0/32/64 only; DVE instructions read at most one PSUM operand.

I/O: ptsn ships unreplicated and is expanded on-device by a
partition-broadcast load DMA; the output is written bf16 (host upcasts
after the gather).  The prologue xn relus alternate Act/DVE (DVE is
idle until the first gate weights arrive).

Dispatch-path packing: the per-dispatch cost of this environment is
dominated by per-tensor and per-byte I/O overhead, not device compute,
so ALL inputs ship as ONE 1-D int8 blob per core (feats quantized to
int8 with an MSE-optimal clip, scale folded into w0ddT on the host;
pt80/consts ride as raw bitcast bytes).  feats are converted int8->bf16
on device (split across Pool and Act) before the xn matmuls.
"""

import numpy as np

B, N, K = 8, 8192, 10
CH = 64          # IN == MID == OUT
SP = 8
GN = CH // SP    # 8 gate channels
TN = 512         # points per tile
TN2 = TN // 2    # points per partition-half
NT = N // TN     # 16 tiles

# packed bf16 const layout: name -> (rows, cols, col offset)
_BF16_LAYOUT = {}
_off = 0
for _name, _r, _c in [
    ("w0ddT", 128, 128), ("w3ddT", 128, 128), ("dls", 128, 160),
    ("w1vA", 10, 16), ("w1vB", 10, 16),
    ("ww2A", 16, 80), ("ww2B", 16, 80), ("s8selA", 80, 16),
    ("vselA", 80, 128), ("vselB", 80, 128),
    ("woutddT", 128, 128), ("idd", 128, 128), ("boutrow", 1, 128),
    ("ones1", 1, 256), ("obcb", 16, 128),
]:
    _BF16_LAYOUT[_name] = (_r, _c, _off)
    _off += _c
_BF16_W = _off

_F8_LAYOUT = {}
_off = 0
for _name, _r, _c in [
    ("s8selA", 80, 16), ("s8selB", 80, 16), ("wrep", 80, 640),
]:
    _F8_LAYOUT[_name] = (_r, _c, _off)
    _off += _c
_F8_W = _off

_F32_LAYOUT = {}
_off = 0
for _name, _r, _c in [
    ("b0dd", 128, 1), ("cbdd", 16, 1), ("b3dd", 128, 1),
    ("bw2col", 80, 1), ("boutdd", 128, 1),
]:
    _F32_LAYOUT[_name] = (_r, _c, _off)
    _off += _c
_F32_W = _off

# single-blob byte layout (all regions 512B-aligned)
OFF_FEATS = 0
FEATS_BYTES = 128 * (N * K // 2)             # int8, k-major packed
OFF_PT80 = OFF_FEATS + FEATS_BYTES
PT80_BYTES = NT * K * 2 * TN2 * 2            # bf16 [NT, K, 2*TN2]
OFF_CB = OFF_PT80 + PT80_BYTES
CB_BYTES = 128 * _BF16_W * 2
OFF_CF = OFF_CB + CB_BYTES
CF_BYTES = 128 * _F32_W * 4
OFF_C8 = OFF_CF + CF_BYTES
C8_BYTES = 128 * _F8_W
TOTAL_BYTES = OFF_C8 + C8_BYTES
assert all(o % 512 == 0 for o in (OFF_PT80, OFF_CB, OFF_CF, OFF_C8))

_CACHE = {}


def _build_bass():
    import concourse.bacc as bacc
    import concourse.tile as tile
    from concourse import mybir

    f32 = mybir.dt.float32
    f32r = mybir.dt.float32r
    bf16 = mybir.dt.bfloat16
    f8 = mybir.dt.float8e4
    i8 = mybir.dt.int8
    AF = mybir.ActivationFunctionType
    OP = mybir.AluOpType

    nc = bacc.Bacc("TRN2", target_bir_lowering=False)

    def mm(out, lhsT, rhs, **kw):
        nc.tensor.matmul(out, lhsT, rhs, **kw)

    # ---------------- DRAM I/O ----------------
    # ONE packed input blob per core: the dispatch path charges ~1.5ms
    # per external tensor per iteration plus a per-byte toll, so all
    # inputs ride in a single 1-D int8 tensor and are bitcast on access.
    blob_d = nc.dram_tensor("blob", [TOTAL_BYTES], i8,
                            kind="ExternalInput")
    # feats pre-packed on host: [c + 64*half, ((tile, k), within)] k-major,
    # quantized int8 (scale folded into w0ddT host-side)
    feats_v = blob_d[OFF_FEATS:OFF_FEATS + FEATS_BYTES] \
        .rearrange("(p x) -> p x", p=128)
    # ptsn per tile, [tile, k, h*256+n]; the 8-group replication happens
    # in the load DMA via a partition-broadcast access pattern
    pt80_v = blob_d[OFF_PT80:OFF_PT80 + PT80_BYTES].bitcast(bf16) \
        .rearrange("(t k n) -> t k n", t=NT, k=K)
    cpack_b_v = blob_d[OFF_CB:OFF_CB + CB_BYTES].bitcast(bf16) \
        .rearrange("(p x) -> p x", p=128)
    cpack_f_v = blob_d[OFF_CF:OFF_CF + CF_BYTES].bitcast(f32) \
        .rearrange("(p x) -> p x", p=128)
    cpack_8_v = blob_d[OFF_C8:OFF_C8 + C8_BYTES].bitcast(f8) \
        .rearrange("(p x) -> p x", p=128)
    out_d = nc.dram_tensor("out", [128, N // 2], bf16,
                           kind="ExternalOutput")

    with tile.TileContext(nc) as tc:
        with (
            tc.tile_pool(name="singles", bufs=1) as singles,
            tc.tile_pool(name="bigio", bufs=3) as bigio,
            tc.tile_pool(name="fconv", bufs=2) as fconv,
            tc.tile_pool(name="xnpool", bufs=4) as xnpool,
            tc.tile_pool(name="p80pool", bufs=4) as p80pool,
            tc.tile_pool(name="work", bufs=3) as work,
            tc.tile_pool(name="small", bufs=3) as small,
            tc.tile_pool(name="xnps", bufs=2, space="PSUM") as xnps_pool,
            tc.tile_pool(name="x3ps", bufs=3, space="PSUM") as x3ps_pool,
            tc.tile_pool(name="sbank", bufs=1, space="PSUM") as sbank_pool,
        ):
            # ---- packed consts in SBUF (three DMAs from the blob) ----
            cpack_b = singles.tile([128, _BF16_W], bf16, name="cpack_b")
            nc.sync.dma_start(out=cpack_b, in_=cpack_b_v[:, :])
            cpack_f = singles.tile([128, _F32_W], f32, name="cpack_f")
            nc.sync.dma_start(out=cpack_f, in_=cpack_f_v[:, :])
            cpack_8 = singles.tile([128, _F8_W], f8, name="cpack_8")
            nc.sync.dma_start(out=cpack_8, in_=cpack_8_v[:, :])
            csb = {}
            for name, (r, c, off) in _BF16_LAYOUT.items():
                csb[name] = cpack_b[0:r, off:off + c]
            for name, (r, c, off) in _F32_LAYOUT.items():
                csb[name] = cpack_f[0:r, off:off + c]
            for name, (r, c, off) in _F8_LAYOUT.items():
                csb[name] = cpack_8[0:r, off:off + c]

            # persistent small PSUM banks; matmul PSUM writes must start at
            # partition 0/32/64.  wv (80 rows, consumed early each tile by
            # the e exp) rides the x3 pool rotation instead of owning a bank.
            SB1 = sbank_pool.tile([128, 512], f32, name="SB1")
            SB3a = sbank_pool.tile([128, 512], f32, name="SB3a")
            SB3b = sbank_pool.tile([128, 512], f32, name="SB3b")
            G_ps = SB1[0:16, 0:TN2]
            s8_ps = SB1[0:16, TN2:2 * TN2]
            # rsb/P double-buffered by tile parity: the tile's tail (stt
            # reads) is deferred past the NEXT tile's k-group matmuls
            rsb_bank = [SB3a[0:128, 0:TN2], SB3b[0:128, 0:TN2]]
            P_bank = [SB3a[:, TN2:2 * TN2], SB3b[:, TN2:2 * TN2]]

            KGROUPS = [(0, 2), (2, 2), (4, 2), (6, 2), (8, 2)]

            def dma_inputs(it):
                ft8 = bigio.tile([128, K * TN2], i8, name="feats8_t")
                nc.sync.dma_start(
                    out=ft8, in_=feats_v[:, it * TN2 * K:(it + 1) * TN2 * K])
                p80 = p80pool.tile([80, 2 * TN2], bf16, name="pt80")
                nc.sync.dma_start(out=p80,
                                  in_=pt80_v[it].partition_broadcast(SP))
                return ft8, p80

            def conv_feats(ft8):
                # int8 -> bf16 (exact) for the PE; split Pool/Act so
                # neither engine eats the whole 2560 el/partition
                ftb = fconv.tile([128, K * TN2], bf16, name="featsb_t")
                hw = K * TN2 // 2
                nc.gpsimd.tensor_copy(out=ftb[:, 0:hw], in_=ft8[:, 0:hw])
                nc.scalar.copy(out=ftb[:, hw:], in_=ft8[:, hw:])
                return ftb

            def xn_chunk(xn_sb, feats_t, ci, dve=False):
                xn_ps = xnps_pool.tile([128, 512], f32, name="xn_ps",
                                       tag="xnps")
                mm(xn_ps, csb["w0ddT"], feats_t[:, ci * 512:(ci + 1) * 512],
                   start=True, stop=True)
                dst = xn_sb[:, 2 * ci:2 * ci + 2, :].rearrange(
                    "p a n -> p (a n)")
                if dve:
                    # prologue only: DVE is idle during pipeline fill
                    nc.vector.tensor_scalar(
                        out=dst, in0=xn_ps, scalar1=csb["b0dd"], scalar2=0.0,
                        op0=OP.add, op1=OP.max)
                else:
                    nc.scalar.activation(
                        out=dst, in_=xn_ps, func=AF.Relu, bias=csb["b0dd"])

            def gate_chain(it, xn_sb, pt80_sb):  # noqa: returns e, ws
                """G -> h -> e logits -> exp -> PE selector replication of
                the per-channel gate weights for tile `it`."""
                for j in range(K):
                    mm(G_ps, csb["dls"][:, 16 * j:16 * (j + 1)],
                       xn_sb[:, j, :], start=(j == 0), stop=False)
                mm(G_ps, csb["w1vA"], pt80_sb[0:K, 0:TN2], start=False,
                   stop=False)
                mm(G_ps, csb["w1vB"], pt80_sb[0:K, TN2:2 * TN2], start=False,
                   stop=True)
                h_sb = work.tile([16, TN2], bf16, name="h_sb")
                nc.scalar.activation(
                    out=h_sb, in_=G_ps, func=AF.Relu, bias=csb["cbdd"])
                e_sb = work.tile([80, 512], f8, name="e_sb")
                wv_t = xnps_pool.tile([128, 512], f32, name="wv_ps",
                                      tag="xnps")
                wv_ps = wv_t[0:80, :]
                mm(wv_ps[:, 0:TN2], csb["ww2A"], h_sb, start=True, stop=True)
                mm(wv_ps[:, TN2:2 * TN2], csb["ww2B"], h_sb, start=True,
                   stop=True)
                nc.scalar.activation(
                    out=e_sb, in_=wv_ps, func=AF.Exp, bias=csb["bw2col"])
                # per-channel gate weights ws[c+64h, k, n] = e[(c%8)*10+k,
                # h*256+n]: a pure partition replication of e.  Engines
                # cannot replicate partitions but PE selector matmuls can:
                # per k, two [80, 64] 0/1-stationary matmuls place the two
                # point-halves on partitions 0:64 / 64:128 of one PSUM
                # half-bank, then one Act copy per k-pair lands f8 ws_sb.
                # (All deps SBUF/PSUM-tracked -- no HBM round trip, whose
                # write->gather ordering relied on per-queue DMA FIFO that
                # HWDGE does not guarantee across transfer shapes.)
                ws_sb = work.tile([128, K, TN2], f8, name="ws_sb")
                for kp in range(K // 2):
                    wr = xnps_pool.tile([128, 512], f32, name="wsrep_ps",
                                        tag="xnps")
                    for dk in range(2):
                        k = 2 * kp + dk
                        rk = csb["wrep"][:, 64 * k:64 * k + 64]
                        mm(wr[0:64, dk * TN2:(dk + 1) * TN2], rk,
                           e_sb[:, 0:TN2], start=True, stop=True)
                        mm(wr[64:128, dk * TN2:(dk + 1) * TN2], rk,
                           e_sb[:, TN2:2 * TN2], start=True, stop=True)
                    nc.scalar.copy(
                        out=ws_sb[:, 2 * kp:2 * kp + 2, :].rearrange(
                            "p a n -> p (a n)"), in_=wr)
                return e_sb, ws_sb

            # ---- prologue: 3 tiles of inputs, xn(0..1), gate chains ----
            feats_cur, pt80_cur = dma_inputs(0)
            feats_nxt, pt80_nxt = dma_inputs(1)
            feats_n2, pt80_n2 = dma_inputs(2)
            xn_cur = xnpool.tile([128, K, TN2], bf16, name="xn_sb")
            ftb = conv_feats(feats_cur)
            for ci in range(5):
                xn_chunk(xn_cur, ftb, ci, dve=(ci % 2 == 1))
            xn_nxt = xnpool.tile([128, K, TN2], bf16, name="xn_sb")
            ftb = conv_feats(feats_nxt)
            for ci in range(5):
                xn_chunk(xn_nxt, ftb, ci, dve=(ci % 2 == 1))
            e_cur, ws_cur = gate_chain(0, xn_cur, pt80_cur)
            e_nxt, ws_nxt = gate_chain(1, xn_nxt, pt80_nxt)

            pending_tail = None
            for it in range(NT):
                xn_sb, pt80_sb = xn_cur, pt80_cur
                e_sb, ws_sb = e_cur, ws_cur
                build = it + 2 < NT            # build tile it+2 this iter
                if it + 3 < NT:
                    feats_n3, pt80_n3 = dma_inputs(it + 3)
                if build:
                    xn_n2 = xnpool.tile([128, K, TN2], bf16, name="xn_sb")
                    ftb_n2 = conv_feats(feats_n2)

                # gated-ptsn product in 80-row space; its k-sum happens
                # inside the vsel broadcast matmuls below
                m80_sb = work.tile([80, 2 * TN2], bf16, name="m80_sb")
                nc.vector.tensor_tensor(
                    out=m80_sb, in0=e_sb, in1=pt80_sb, op=OP.mult)

                # ---------- per-k-group: x3 matmul + gate product;
                # tile it+2's xn chunks fill the PE gaps ----------
                y_sb = work.tile([128, K, TN2], bf16, name="y_sb")
                rs8_sb = small.tile([16, TN2], f32, name="rs8_sb")
                for gi, (k0, kg) in enumerate(KGROUPS):
                    x3_ps = x3ps_pool.tile([128, 2, TN2], f32, name="x3_ps",
                                           tag="x3")
                    mm(x3_ps[:, 0:kg, :].rearrange("p a n -> p (a n)"),
                       csb["w3ddT"],
                       xn_sb[:, k0:k0 + kg, :].rearrange("p a n -> p (a n)"),
                       start=True, stop=True)
                    nc.vector.tensor_tensor(
                        out=y_sb[:, k0:k0 + kg, :],
                        in0=ws_sb[:, k0:k0 + kg, :], in1=x3_ps[:, 0:kg, :],
                        op=OP.mult)
                    if build:
                        xn_chunk(xn_n2, ftb_n2, gi)
                    if gi == 0:
                        mm(s8_ps, csb["s8selA"], e_sb[:, 0:TN2], start=True,
                           stop=False)
                        mm(s8_ps, csb["s8selB"], e_sb[:, TN2:2 * TN2],
                           start=False, stop=True)
                        nc.vector.reciprocal_approx_fast(
                            out=rs8_sb, in_=s8_ps)
                        rs8b_sb = small.tile([16, TN2], bf16, name="rs8b_sb")
                        nc.gpsimd.tensor_copy(out=rs8b_sb, in_=rs8_sb)
                    elif gi == 1:
                        mm(rsb_bank[it % 2], csb["obcb"], rs8b_sb,
                           start=True, stop=True)
                    elif gi == 2:
                        mm(P_bank[it % 2], csb["vselA"], m80_sb[:, 0:TN2],
                           start=True, stop=False)
                        mm(P_bank[it % 2], csb["vselB"],
                           m80_sb[:, TN2:2 * TN2], start=False, stop=True)

                # ---------- weighted sum over k (tree) ----------
                nc.gpsimd.tensor_tensor(
                    out=y_sb[:, 0:5, :], in0=y_sb[:, 0:5, :],
                    in1=y_sb[:, 5:10, :], op=OP.add)
                nc.gpsimd.tensor_tensor(
                    out=y_sb[:, 0:2, :], in0=y_sb[:, 0:2, :],
                    in1=y_sb[:, 2:4, :], op=OP.add)
                t01_sb = small.tile([128, TN2], bf16, name="t01_sb")
                nc.gpsimd.tensor_tensor(
                    out=t01_sb, in0=y_sb[:, 0, :], in1=y_sb[:, 1, :],
                    op=OP.add)
                num_sb = small.tile([128, TN2], bf16, name="num_sb")
                nc.gpsimd.tensor_tensor(
                    out=num_sb, in0=t01_sb, in1=y_sb[:, 4, :], op=OP.add)

                # tail of the previous tile rides here, after this tile's
                # mults and tree are queued
                if pending_tail is not None:
                    pending_tail()
                    pending_tail = None

                def tail(num_sb=num_sb, xn_sb=xn_sb, it=it):
                    # normalize + relu + output head for tile `it`, deferred
                    # past tile it+1's k-group mults so the Pool-tree wait
                    # doesn't block DVE's queue between tiles.  (HW allows
                    # at most one PSUM input per DVE instruction, so the
                    # P-add and rsb-mult stay two separate stts.)
                    num2_sb = small.tile([128, TN2], bf16, name="num2_sb")
                    nc.vector.scalar_tensor_tensor(
                        out=num2_sb, in0=P_bank[it % 2], scalar=0.0,
                        in1=num_sb, op0=OP.bypass, op1=OP.add)
                    o1p_sb = small.tile([128, TN2], bf16, name="o1p_sb")
                    nc.vector.scalar_tensor_tensor(
                        out=o1p_sb, in0=num2_sb, scalar=0.0,
                        in1=rsb_bank[it % 2], op0=OP.bypass, op1=OP.mult)
                    o1_sb = small.tile([128, TN2], bf16, name="o1_sb")
                    nc.gpsimd.tensor_scalar(
                        out=o1_sb, in0=o1p_sb, scalar1=csb["b3dd"],
                        scalar2=0.0, op0=OP.add, op1=OP.max)
                    out2_ps = P_bank[it % 2]
                    mm(out2_ps, csb["woutddT"], o1_sb, start=True, stop=False)
                    mm(out2_ps, csb["idd"], xn_sb[:, 0, :], start=False,
                       stop=False)
                    mm(out2_ps, csb["boutrow"], csb["ones1"], start=False,
                       stop=True)
                    fin_sb = small.tile([128, TN2], bf16, name="fin_sb")
                    nc.scalar.copy(out=fin_sb, in_=out2_ps)
                    nc.sync.dma_start(
                        out=out_d[:, it * TN2:(it + 1) * TN2], in_=fin_sb)
                pending_tail = tail

                # ---------- gate chain two tiles ahead ----------
                if build:
                    e_n2, ws_n2 = gate_chain(it + 2, xn_n2, pt80_n2)
                    xn_cur, xn_nxt = xn_nxt, xn_n2
                    e_cur, ws_cur = e_nxt, ws_nxt
                    e_nxt, ws_nxt = e_n2, ws_n2
                    feats_cur, pt80_cur = feats_nxt, pt80_nxt
                    feats_nxt, pt80_nxt = feats_n2, pt80_n2
                    if it + 3 < NT:
                        feats_n2, pt80_n2 = feats_n3, pt80_n3
                elif it + 1 < NT:
                    xn_cur = xn_nxt
                    e_cur, ws_cur = e_nxt, ws_nxt
                    feats_cur, pt80_cur = feats_nxt, pt80_nxt

            pending_tail()

    nc.compile()
    return nc


def _fold_weights(inp):
    """Host-side weight folding -> dict of const arrays (f32)."""
    W0, b0 = inp["W0"], inp["b0"]
    W1, b1 = inp["W1"], inp["b1"]
    W2, b2 = inp["W2"], inp["b2"]
    W3, b3 = inp["W3"], inp["b3"]
    Wp1, Wp2 = inp["Wp1"], inp["Wp2"]
    Ww1, Ww2, bw2 = inp["Ww1"], inp["Ww2"], inp["bw2"]
    Wout, bout = inp["Wout"], inp["bout"]

    Ww1r = Ww1.reshape(GN, CH, K)
    A = Ww1r.sum(axis=2)
    AW1 = A @ W1
    C2 = np.einsum("omj,mc->ocj", Ww1r, W2)
    Dc = -C2.copy()
    Dc[:, :, 0] += AW1
    cb = A @ (b1 - b2)
    v = Wp2 @ np.maximum(Wp1[:, 0], 0.0)
    w1v = np.einsum("omj,m->oj", Ww1r, v)

    m64 = np.arange(CH)

    c = {}
    t = np.zeros((128, 128), np.float32)
    t[0:64, 0:64] = W0.T; t[64:128, 64:128] = W0.T
    c["w0ddT"] = t
    c["b0dd"] = np.concatenate([b0, b0]).reshape(128, 1)
    t = np.zeros((128, 128), np.float32)
    t[0:64, 0:64] = W3.T; t[64:128, 64:128] = W3.T
    c["w3ddT"] = t
    t = np.zeros((128, 10 * 16), np.float32)
    for j in range(K):
        t[0:64, 16 * j:16 * j + 8] = Dc[:, :, j].T
        t[64:128, 16 * j + 8:16 * j + 16] = Dc[:, :, j].T
    c["dls"] = t
    t = np.zeros((K, 16), np.float32)
    for j in range(K):
        t[j, 0:8] = w1v[:, j]
    c["w1vA"] = t
    t = np.zeros((K, 16), np.float32)
    for j in range(K):
        t[j, 8:16] = w1v[:, j]
    c["w1vB"] = t
    c["cbdd"] = np.concatenate([cb, cb]).reshape(16, 1).astype(np.float32)
    # vsel[g*10+k, c + 64h] = v[c] * [g == c % 8]: the per-k gated-ptsn
    # broadcast; contraction over the 80 rows sums over k for free.
    for h, nm in ((0, "vselA"), (1, "vselB")):
        t = np.zeros((80, 128), np.float32)
        for g in range(SP):
            for k in range(K):
                cc = m64[m64 % SP == g]
                t[g * K + k, cc + 64 * h] = v[cc]
        c[nm] = t
    t = np.zeros((16, 80), np.float32); t[0:8, :] = Ww2.T
    c["ww2A"] = t
    t = np.zeros((16, 80), np.float32); t[8:16, :] = Ww2.T
    c["ww2B"] = t
    c["bw2col"] = bw2.reshape(80, 1).astype(np.float32)
    t = np.zeros((80, 16), np.float32)
    for g in range(SP):
        for j in range(K):
            t[g * K + j, g] = 1.0
    c["s8selA"] = t
    t = np.zeros((80, 16), np.float32)
    for g in range(SP):
        for j in range(K):
            t[g * K + j, 8 + g] = 1.0
    c["s8selB"] = t
    # ws replication selectors: block k is [80, 64] with
    # wrep[(c%8)*10 + k, c] = 1 (shared by both point-halves)
    t = np.zeros((80, 640), np.float32)
    for k in range(K):
        for cc in range(64):
            t[(cc % SP) * K + k, 64 * k + cc] = 1.0
    c["wrep"] = t
    t = np.zeros((16, 128), np.float32)
    for h in range(2):
        t[(m64 % SP) + 8 * h, m64 + 64 * h] = 1.0
    c["obcb"] = t
    t = np.zeros((128, 128), np.float32)
    t[0:64, 0:64] = Wout.T; t[64:128, 64:128] = Wout.T
    c["woutddT"] = t
    c["idd"] = np.eye(128, dtype=np.float32)
    c["b3dd"] = np.concatenate([b3, b3]).reshape(128, 1)
    c["boutdd"] = np.concatenate([bout, bout]).reshape(128, 1)
    c["boutrow"] = np.concatenate([bout, bout]).reshape(1, 128)
    c["ones1"] = np.ones((1, TN2), np.float32)
    return c


def make_in_maps(inputs):
    import ml_dtypes
    bf16 = ml_dtypes.bfloat16
    inp = {k: np.ascontiguousarray(np.asarray(v, dtype=np.float32))
           for k, v in inputs.items()}
    consts = _fold_weights(inp)
    cpack_f = np.zeros((128, _F32_W), np.float32)
    for name, (r, c, off) in _F32_LAYOUT.items():
        cpack_f[0:r, off:off + c] = consts[name]
    f8 = ml_dtypes.float8_e4m3
    cpack_8 = np.zeros((128, _F8_W), f8)
    for name, (r, c, off) in _F8_LAYOUT.items():
        cpack_8[0:r, off:off + c] = consts[name].astype(f8)
    cf_bytes = np.frombuffer(cpack_f.tobytes(), np.int8)
    c8_bytes = np.frombuffer(cpack_8.tobytes(), np.int8)
    # host ptsn for all cores at once: [B, N, K]
    cent = inp["cent_pts"]                      # [B, N, 3]
    spt = inp["sm_pts"]                         # [B, 3, N, K]
    ptsn = ((cent.transpose(0, 2, 1)[:, :, :, None] - spt) ** 2).sum(axis=1)
    in_maps = []
    for b in range(B):
        # k-major pack: [64, NT, 2, TN2, K] -> [64, NT, 2, K, TN2]
        ff = inp["sm_feats"][b].reshape(CH, NT, 2, TN2, K)
        ff = ff.transpose(0, 1, 2, 4, 3)        # [64, NT, 2, K, TN2]
        fpk = np.ascontiguousarray(
            np.concatenate([ff[:, :, 0], ff[:, :, 1]], axis=0)
            .reshape(128, N * K // 2))
        # int8 quantization, MSE-optimal clip (coarse subsampled scan);
        # the scale folds into this core's w0ddT below
        amax = float(np.abs(fpk).max())
        sub = fpk.reshape(-1)[::17]
        best_s, best_mse = None, np.inf
        for frac in (0.68, 0.71, 0.74, 0.77, 0.80, 1.0):
            s = frac * amax / 127.0
            qs = np.clip(np.round(sub / s), -127, 127)
            mse = float(((qs * s - sub) ** 2).mean())
            if mse < best_mse:
                best_s, best_mse = s, mse
        fq = np.clip(np.round(fpk / best_s), -127, 127).astype(np.int8)
        cpack_b = np.zeros((128, _BF16_W), bf16)
        for name, (r, c, off) in _BF16_LAYOUT.items():
            arr = consts[name]
            if name == "w0ddT":
                arr = arr * best_s
            cpack_b[0:r, off:off + c] = arr.astype(bf16)
        # pt80[it, k, h*256+n] = ptsn[it, h, n, k]; replicated on-device
        pt80 = np.ascontiguousarray(
            ptsn[b].reshape(NT, 2, TN2, K).transpose(0, 3, 1, 2)
            .reshape(NT, K, 2 * TN2).astype(bf16))
        blob = np.zeros(TOTAL_BYTES, np.int8)
        blob[OFF_FEATS:OFF_FEATS + FEATS_BYTES] = fq.reshape(-1)
        blob[OFF_PT80:OFF_PT80 + PT80_BYTES] = \
            np.frombuffer(pt80.tobytes(), np.int8)
        blob[OFF_CB:OFF_CB + CB_BYTES] = \
            np.frombuffer(cpack_b.tobytes(), np.int8)
        blob[OFF_CF:OFF_CF + CF_BYTES] = cf_bytes
        blob[OFF_C8:OFF_C8 + C8_BYTES] = c8_bytes
        in_maps.append({"blob": blob})
    return in_maps


def _run(inputs, trace=False):
    from concourse.bass_utils import run_bass_kernel_spmd

    if "nc" not in _CACHE:
        _CACHE["nc"] = _build_bass()
    nc = _CACHE["nc"]
    in_maps = make_in_maps(inputs)

    res = run_bass_kernel_spmd(
        nc, in_maps, core_ids=list(range(B)), trace=trace)
    outs = []
    for r in res.results:
        o = np.asarray(r["out"]).astype(np.float32) \
            .reshape(2, CH, NT, TN2)               # [half, c, tile, n]
        outs.append(np.ascontiguousarray(
            o.transpose(1, 2, 0, 3).reshape(CH, N)))
    out = np.stack(outs, axis=0)
    return out, res


def kernel(**inputs) -> np.ndarray:
    out, _ = _run(inputs, trace=False)
    return out



# revision 50
# speedup vs baseline: 2.3723x; 1.4550x over previous
"""Point-Transformer block as a Bass/Tile kernel for 8 Trainium2 NeuronCores.

Strategy
--------
ALL 8 batch elements run on ONE NeuronCore (128 tiles): the dispatch
path charges per-core per-operand overhead that dwarfs device time, so
fewer cores is strictly faster (measured 8c=8.8ms vs 1c=4.0ms per
dispatch at identical total bytes).  Tiles are independent points and
the 1x1-conv weights are shared, so the tile pipeline is batch-agnostic.

Host-side algebraic folding (all on 64x64-ish weights, negligible cost):
  * x1/x2 are never materialized: the gate-logit path folds into per-j
    64->8 matmuls with weights D_j = -Ww1_j@W2 (+ Ww1-rowsum@W1 for j==0).
  * pt_conv collapses: ptsn >= 0 so relu(Wp1*ptsn) = relu(Wp1)*ptsn, hence
    ptf = v (x) ptsn with v = Wp2 @ relu(Wp1); its contribution to the
    gated sum is computed in 80-row (group,k) space and folded into two
    `vsel` broadcast matmuls whose contraction performs the k-sum.
  * b3 is deferred through the softmax (sum_k ws = 1); bout rides a
    ones-row matmul inside the output-head PSUM accumulation.

Layout: 512-point tiles, the two 256-point halves packed on partitions
[0:64)/[64:128) so all 64-channel matmuls run with block-diagonal weights
at full PE occupancy.  feats are bf16 and k-major in HBM so xn PSUM
chunks are exactly [128, 512]; activations/weights are bf16 (matmul cols
then cost 1 PE cycle); the gate weights e are fp8e4 (the softmax
normalization cancels most of the quantization).

The per-channel gate weights ws[c,k,n] = e[(c%8)*10+k,n] are a pure
partition replication of e: engines cannot replicate partitions; a DMA
gather from HBM scratch can.  Ordering is margin-based, NOT ring-FIFO-
based: the scratch write issues with the gate chain two tiles ahead of
consumption, the gather one iteration later (ws_gather), and the slot
rotation is 8 deep, so write->gather and slot-reuse hazards carry >= a
tile period (~10us) and ~80us of slack respectively.

Software pipeline (per iteration): consume tile it (x3 matmuls + DVE
gate products + Pool k-reduction tree), build tile it+2's xn and launch
its gate chain (G -> h -> e -> exp -> PE ws replication) so the chain
latency hides under two tile periods; the normalize/relu/head tail of
tile it-1 is emitted after tile it's mults so its Pool-tree wait never
head-blocks DVE's queue; rsb/P PSUM banks are parity double-buffered.
GPSIMD never touches PSUM (illegal on HW); matmul PSUM writes start at
partition 0/32/64 only; DVE instructions read at most one PSUM operand.

I/O: ptsn ships unreplicated and is expanded on-device by a
partition-broadcast load DMA; the output ships uint8 with a per-row
f32 scale in the last 4 bytes (device computes absmax/reciprocal, host
dequantizes).  The prologue xn relus alternate Act/DVE (DVE is idle
until the first gate weights arrive).

Dispatch-path packing: the per-dispatch cost of this environment is
dominated by per-tensor and per-byte I/O overhead, not device compute,
so ALL inputs ship as ONE 1-D int8 blob per core (feats quantized to
int8 with an MSE-optimal clip, scale folded into w0ddT on the host;
pt80/consts ride as raw bitcast bytes).  feats are converted int8->bf16
on device (split across Pool and Act) before the xn matmuls.
"""

import numpy as np

B = 1            # NeuronCores used (dispatch overhead is per-core, so
                 # all 8 batch elements run on ONE core: tiles are
                 # independent points and the weights are shared)
NBATCH = 8       # batch elements, all on core 0
N, K = 8192, 10  # points per batch element
CH = 64          # IN == MID == OUT
SP = 8
GN = CH // SP    # 8 gate channels
TN = 512         # points per tile
TN2 = TN // 2    # points per partition-half
NTB = N // TN    # 16 tiles per batch element
NT = NBATCH * NTB                # 128 tiles total
ESCR = 8         # e-scratch rotation depth (slot reuse ~80us apart)
OUTC = NBATCH * (N // 2)         # 32768 uint8 data cols
OUTW = OUTC + 4 * NBATCH         # + per-batch f32 row scales

# packed bf16 const layout: name -> (rows, cols, col offset)
_BF16_LAYOUT = {}
_off = 0
for _name, _r, _c in [
    ("w0ddT", 128, 128), ("w3ddT", 128, 128), ("dls", 128, 160),
    ("w1vA", 10, 16), ("w1vB", 10, 16),
    ("ww2A", 16, 80), ("ww2B", 16, 80), ("s8selA", 80, 16),
    ("vselA", 80, 128), ("vselB", 80, 128),
    ("woutddT", 128, 128), ("idd", 128, 128), ("boutrow", 1, 128),
    ("ones1", 1, 256), ("obcb", 16, 128),
]:
    _BF16_LAYOUT[_name] = (_r, _c, _off)
    _off += _c
_BF16_W = _off

_F8_LAYOUT = {}
_off = 0
for _name, _r, _c in [
    ("s8selA", 80, 16), ("s8selB", 80, 16), ("wrep", 80, 640),
]:
    _F8_LAYOUT[_name] = (_r, _c, _off)
    _off += _c
_F8_W = _off

_F32_LAYOUT = {}
_off = 0
for _name, _r, _c in [
    ("b0dd", 128, 1), ("cbdd", 16, 1), ("b3dd", 128, 1),
    ("bw2col", 80, 1), ("boutdd", 128, 1),
]:
    _F32_LAYOUT[_name] = (_r, _c, _off)
    _off += _c
_F32_W = _off

# single-blob byte layout (all regions 512B-aligned)
OFF_FEATS = 0
FEATS_BYTES = 128 * (NBATCH * N * K // 2)    # int8, k-major packed
OFF_PT80 = OFF_FEATS + FEATS_BYTES
PT80_BYTES = NT * K * 2 * TN2 * 2            # bf16 [NT, K, 2*TN2]
OFF_CB = OFF_PT80 + PT80_BYTES
CB_BYTES = 128 * _BF16_W * 2
OFF_CF = OFF_CB + CB_BYTES
CF_BYTES = 128 * _F32_W * 4
OFF_C8 = OFF_CF + CF_BYTES
C8_BYTES = 128 * _F8_W
TOTAL_BYTES = OFF_C8 + C8_BYTES
assert all(o % 512 == 0 for o in (OFF_PT80, OFF_CB, OFF_CF, OFF_C8))

_CACHE = {}


def _build_bass():
    import concourse.bacc as bacc
    import concourse.tile as tile
    from concourse import mybir

    f32 = mybir.dt.float32
    f32r = mybir.dt.float32r
    bf16 = mybir.dt.bfloat16
    f8 = mybir.dt.float8e4
    i8 = mybir.dt.int8
    AF = mybir.ActivationFunctionType
    OP = mybir.AluOpType

    # enable_partition_id=False drops the implicit [1,1] partition_id
    # ExternalInput: the dispatch path charges per OPERAND, and this kernel
    # never branches on core id (per-core behavior lives in the blob data)
    nc = bacc.Bacc("TRN2", target_bir_lowering=False,
                   enable_partition_id=False)

    def mm(out, lhsT, rhs, **kw):
        nc.tensor.matmul(out, lhsT, rhs, **kw)

    # ---------------- DRAM I/O ----------------
    # ONE packed input blob per core: the dispatch path charges ~1.5ms
    # per external tensor per iteration plus a per-byte toll, so all
    # inputs ride in a single 1-D int8 tensor and are bitcast on access.
    blob_d = nc.dram_tensor("blob", [TOTAL_BYTES], i8,
                            kind="ExternalInput")
    # feats pre-packed on host: [c + 64*half, ((batch, tile, k), within)]
    # k-major, quantized int8 (one global scale folded into w0ddT)
    feats_v = blob_d[OFF_FEATS:OFF_FEATS + FEATS_BYTES] \
        .rearrange("(p x) -> p x", p=128)
    # ptsn per tile, [tile, k, h*256+n]; the 8-group replication happens
    # in the load DMA via a partition-broadcast access pattern
    pt80_v = blob_d[OFF_PT80:OFF_PT80 + PT80_BYTES].bitcast(bf16) \
        .rearrange("(t k n) -> t k n", t=NT, k=K)
    cpack_b_v = blob_d[OFF_CB:OFF_CB + CB_BYTES].bitcast(bf16) \
        .rearrange("(p x) -> p x", p=128)
    cpack_f_v = blob_d[OFF_CF:OFF_CF + CF_BYTES].bitcast(f32) \
        .rearrange("(p x) -> p x", p=128)
    cpack_8_v = blob_d[OFF_C8:OFF_C8 + C8_BYTES].bitcast(f8) \
        .rearrange("(p x) -> p x", p=128)
    # output ships uint8 (per-batch per-row f32 scales in the last 32
    # bytes): the dispatch path charges ~1ms/MB on the output round
    # trip, and engine f32->uint8 conversion is round-half-even with
    # saturation, so q = convert(x*r + 128) is exact and overflow-safe
    out_d = nc.dram_tensor("out", [128, OUTW], mybir.dt.uint8,
                           kind="ExternalOutput")
    # HBM staging for the gate-weight broadcast: written by gate_chain
    # (2 tiles ahead of use), gathered by ws_gather (1 tile ahead), so
    # every ordering has >= a tile period of margin on one SWDGE ring
    e_scr = nc.dram_tensor("e_scr", [ESCR, 2, 80, TN2], f8,
                           kind="Internal")

    with tile.TileContext(nc) as tc:
        with (
            tc.tile_pool(name="singles", bufs=1) as singles,
            tc.tile_pool(name="bigio", bufs=3) as bigio,
            tc.tile_pool(name="fconv", bufs=2) as fconv,
            tc.tile_pool(name="xnpool", bufs=4) as xnpool,
            tc.tile_pool(name="p80pool", bufs=4) as p80pool,
            tc.tile_pool(name="work", bufs=3) as work,
            tc.tile_pool(name="small", bufs=3) as small,
            tc.tile_pool(name="xnps", bufs=2, space="PSUM") as xnps_pool,
            tc.tile_pool(name="x3ps", bufs=3, space="PSUM") as x3ps_pool,
            tc.tile_pool(name="sbank", bufs=1, space="PSUM") as sbank_pool,
        ):
            # ---- packed consts in SBUF (three DMAs from the blob) ----
            cpack_b = singles.tile([128, _BF16_W], bf16, name="cpack_b")
            nc.sync.dma_start(out=cpack_b, in_=cpack_b_v[:, :])
            cpack_f = singles.tile([128, _F32_W], f32, name="cpack_f")
            nc.sync.dma_start(out=cpack_f, in_=cpack_f_v[:, :])
            cpack_8 = singles.tile([128, _F8_W], f8, name="cpack_8")
            nc.sync.dma_start(out=cpack_8, in_=cpack_8_v[:, :])
            csb = {}
            for name, (r, c, off) in _BF16_LAYOUT.items():
                csb[name] = cpack_b[0:r, off:off + c]
            for name, (r, c, off) in _F32_LAYOUT.items():
                csb[name] = cpack_f[0:r, off:off + c]
            for name, (r, c, off) in _F8_LAYOUT.items():
                csb[name] = cpack_8[0:r, off:off + c]

            # persistent small PSUM banks; matmul PSUM writes must start at
            # partition 0/32/64.  wv (80 rows, consumed early each tile by
            # the e exp) rides the x3 pool rotation instead of owning a bank.
            fin_seg = singles.tile([128, N // 2], bf16, name="fin_seg")
            u8a = singles.tile([128, OUTC], mybir.dt.uint8, name="u8a")
            scales_sb = singles.tile([128, NBATCH], f32, name="scales_sb")

            SB1 = sbank_pool.tile([128, 512], f32, name="SB1")
            SB3a = sbank_pool.tile([128, 512], f32, name="SB3a")
            SB3b = sbank_pool.tile([128, 512], f32, name="SB3b")
            G_ps = SB1[0:16, 0:TN2]
            s8_ps = SB1[0:16, TN2:2 * TN2]
            # rsb/P double-buffered by tile parity: the tile's tail (stt
            # reads) is deferred past the NEXT tile's k-group matmuls
            rsb_bank = [SB3a[0:128, 0:TN2], SB3b[0:128, 0:TN2]]
            P_bank = [SB3a[:, TN2:2 * TN2], SB3b[:, TN2:2 * TN2]]

            KGROUPS = [(0, 2), (2, 2), (4, 2), (6, 2), (8, 2)]

            def dma_inputs(it):
                ft8 = bigio.tile([128, K * TN2], i8, name="feats8_t")
                nc.sync.dma_start(
                    out=ft8, in_=feats_v[:, it * TN2 * K:(it + 1) * TN2 * K])
                p80 = p80pool.tile([80, 2 * TN2], bf16, name="pt80")
                nc.sync.dma_start(out=p80,
                                  in_=pt80_v[it].partition_broadcast(SP))
                return ft8, p80

            def conv_feats(ft8):
                # int8 -> bf16 (exact) for the PE; split Pool/Act so
                # neither engine eats the whole 2560 el/partition
                ftb = fconv.tile([128, K * TN2], bf16, name="featsb_t")
                hw = K * TN2 // 2
                nc.gpsimd.tensor_copy(out=ftb[:, 0:hw], in_=ft8[:, 0:hw])
                nc.scalar.copy(out=ftb[:, hw:], in_=ft8[:, hw:])
                return ftb

            def xn_chunk(xn_sb, feats_t, ci, dve=False):
                xn_ps = xnps_pool.tile([128, 512], f32, name="xn_ps",
                                       tag="xnps")
                mm(xn_ps, csb["w0ddT"], feats_t[:, ci * 512:(ci + 1) * 512],
                   start=True, stop=True)
                dst = xn_sb[:, 2 * ci:2 * ci + 2, :].rearrange(
                    "p a n -> p (a n)")
                if dve:
                    # prologue only: DVE is idle during pipeline fill
                    nc.vector.tensor_scalar(
                        out=dst, in0=xn_ps, scalar1=csb["b0dd"], scalar2=0.0,
                        op0=OP.add, op1=OP.max)
                else:
                    nc.scalar.activation(
                        out=dst, in_=xn_ps, func=AF.Relu, bias=csb["b0dd"])

            def gate_chain(it, xn_sb, pt80_sb):  # noqa: returns e, ws
                """G -> h -> e logits -> exp -> PE selector replication of
                the per-channel gate weights for tile `it`."""
                for j in range(K):
                    mm(G_ps, csb["dls"][:, 16 * j:16 * (j + 1)],
                       xn_sb[:, j, :], start=(j == 0), stop=False)
                mm(G_ps, csb["w1vA"], pt80_sb[0:K, 0:TN2], start=False,
                   stop=False)
                mm(G_ps, csb["w1vB"], pt80_sb[0:K, TN2:2 * TN2], start=False,
                   stop=True)
                h_sb = work.tile([16, TN2], bf16, name="h_sb")
                nc.scalar.activation(
                    out=h_sb, in_=G_ps, func=AF.Relu, bias=csb["cbdd"])
                e_sb = work.tile([80, 512], f8, name="e_sb")
                wv_t = xnps_pool.tile([128, 512], f32, name="wv_ps",
                                      tag="xnps")
                wv_ps = wv_t[0:80, :]
                mm(wv_ps[:, 0:TN2], csb["ww2A"], h_sb, start=True, stop=True)
                mm(wv_ps[:, TN2:2 * TN2], csb["ww2B"], h_sb, start=True,
                   stop=True)
                nc.scalar.activation(
                    out=e_sb, in_=wv_ps, func=AF.Exp, bias=csb["bw2col"])
                # stage e to HBM scratch for the partition-replicated ws
                # gather (issued one iteration LATER by ws_gather, so the
                # write has a full tile period to complete -- no DMA-ring
                # FIFO assumption; slot rotation depth 8 gives the reuse
                # hazard ~80us of margin)
                sc = e_scr[it % ESCR]
                nc.gpsimd.dma_start(out=sc[0], in_=e_sb[:, 0:TN2])
                nc.gpsimd.dma_start(out=sc[1], in_=e_sb[:, TN2:2 * TN2])
                return e_sb

            def ws_gather(it):
                # per-channel gate weights ws[c+64h, k, n] = e[(c%8)*10+k,
                # h*256+n]: engines cannot replicate partitions; the DMA
                # gather can (stride-0 partition dim on the DRAM source)
                ws_sb = work.tile([128, K, TN2], f8, name="ws_sb")
                sc = e_scr[it % ESCR]
                for h in (0, 1):
                    wsrc = sc[h].rearrange("(g k) n -> g k n", g=SP) \
                        .partition_broadcast(SP)
                    nc.gpsimd.dma_start(
                        out=ws_sb[64 * h:64 * h + 64, :, :], in_=wsrc)
                return ws_sb

            def quantize_batch(b):
                # batch b's 16 tile tails are all in fin_seg: per-row
                # absmax -> r = 127/amax -> uint8 quantize into u8a
                amax = small.tile([128, 1], f32, name="amax")
                nc.vector.tensor_reduce(
                    out=amax, in_=fin_seg, axis=mybir.AxisListType.X,
                    op=OP.max, apply_absolute_value=True)
                nc.vector.tensor_scalar(
                    out=amax, in0=amax, scalar1=1e-20, scalar2=1.0 / 127.0,
                    op0=OP.max, op1=OP.mult)       # guarded amax/127
                rscr = small.tile([128, 1], f32, name="rscr")
                nc.vector.reciprocal_approx_accurate(
                    out=scales_sb[:, b:b + 1], in_=amax, scratch=rscr)
                nc.vector.tensor_scalar(
                    out=u8a[:, b * (N // 2):(b + 1) * (N // 2)],
                    in0=fin_seg, scalar1=scales_sb[:, b:b + 1],
                    scalar2=128.0, op0=OP.mult, op1=OP.add)

            # ---- prologue: 3 tiles of inputs, xn(0..1), gate chains ----
            feats_cur, pt80_cur = dma_inputs(0)
            feats_nxt, pt80_nxt = dma_inputs(1)
            feats_n2, pt80_n2 = dma_inputs(2)
            xn_cur = xnpool.tile([128, K, TN2], bf16, name="xn_sb")
            ftb = conv_feats(feats_cur)
            for ci in range(5):
                xn_chunk(xn_cur, ftb, ci, dve=(ci % 2 == 1))
            xn_nxt = xnpool.tile([128, K, TN2], bf16, name="xn_sb")
            ftb = conv_feats(feats_nxt)
            for ci in range(5):
                xn_chunk(xn_nxt, ftb, ci, dve=(ci % 2 == 1))
            e_cur = gate_chain(0, xn_cur, pt80_cur)
            e_nxt = gate_chain(1, xn_nxt, pt80_nxt)
            ws_cur = ws_gather(0)
            ws_nxt = None                 # gathered during iteration 0

            pending_tail = None
            for it in range(NT):
                xn_sb, pt80_sb = xn_cur, pt80_cur
                e_sb, ws_sb = e_cur, ws_cur
                build = it + 2 < NT            # build tile it+2 this iter
                if it + 3 < NT:
                    feats_n3, pt80_n3 = dma_inputs(it + 3)
                if build:
                    xn_n2 = xnpool.tile([128, K, TN2], bf16, name="xn_sb")
                    ftb_n2 = conv_feats(feats_n2)

                # gated-ptsn product in 80-row space; its k-sum happens
                # inside the vsel broadcast matmuls below
                m80_sb = work.tile([80, 2 * TN2], bf16, name="m80_sb")
                nc.vector.tensor_tensor(
                    out=m80_sb, in0=e_sb, in1=pt80_sb, op=OP.mult)

                # ---------- per-k-group: x3 matmul + gate product;
                # tile it+2's xn chunks fill the PE gaps ----------
                y_sb = work.tile([128, K, TN2], bf16, name="y_sb")
                rs8_sb = small.tile([16, TN2], f32, name="rs8_sb")
                for gi, (k0, kg) in enumerate(KGROUPS):
                    x3_ps = x3ps_pool.tile([128, 2, TN2], f32, name="x3_ps",
                                           tag="x3")
                    mm(x3_ps[:, 0:kg, :].rearrange("p a n -> p (a n)"),
                       csb["w3ddT"],
                       xn_sb[:, k0:k0 + kg, :].rearrange("p a n -> p (a n)"),
                       start=True, stop=True)
                    nc.vector.tensor_tensor(
                        out=y_sb[:, k0:k0 + kg, :],
                        in0=ws_sb[:, k0:k0 + kg, :], in1=x3_ps[:, 0:kg, :],
                        op=OP.mult)
                    if build:
                        xn_chunk(xn_n2, ftb_n2, gi)
                    if gi == 0:
                        mm(s8_ps, csb["s8selA"], e_sb[:, 0:TN2], start=True,
                           stop=False)
                        mm(s8_ps, csb["s8selB"], e_sb[:, TN2:2 * TN2],
                           start=False, stop=True)
                        nc.vector.reciprocal_approx_fast(
                            out=rs8_sb, in_=s8_ps)
                        rs8b_sb = small.tile([16, TN2], bf16, name="rs8b_sb")
                        nc.gpsimd.tensor_copy(out=rs8b_sb, in_=rs8_sb)
                    elif gi == 1:
                        mm(rsb_bank[it % 2], csb["obcb"], rs8b_sb,
                           start=True, stop=True)
                    elif gi == 2:
                        mm(P_bank[it % 2], csb["vselA"], m80_sb[:, 0:TN2],
                           start=True, stop=False)
                        mm(P_bank[it % 2], csb["vselB"],
                           m80_sb[:, TN2:2 * TN2], start=False, stop=True)

                # ---------- weighted sum over k (tree) ----------
                nc.gpsimd.tensor_tensor(
                    out=y_sb[:, 0:5, :], in0=y_sb[:, 0:5, :],
                    in1=y_sb[:, 5:10, :], op=OP.add)
                nc.gpsimd.tensor_tensor(
                    out=y_sb[:, 0:2, :], in0=y_sb[:, 0:2, :],
                    in1=y_sb[:, 2:4, :], op=OP.add)
                t01_sb = small.tile([128, TN2], bf16, name="t01_sb")
                nc.gpsimd.tensor_tensor(
                    out=t01_sb, in0=y_sb[:, 0, :], in1=y_sb[:, 1, :],
                    op=OP.add)
                num_sb = small.tile([128, TN2], bf16, name="num_sb")
                nc.gpsimd.tensor_tensor(
                    out=num_sb, in0=t01_sb, in1=y_sb[:, 4, :], op=OP.add)

                # gather tile it+1's gate weights (written >= 1 iteration
                # ago, so the scratch write has long since completed)
                if it + 1 < NT:
                    ws_nxt = ws_gather(it + 1)

                # tail of the previous tile rides here, after this tile's
                # mults and tree are queued
                if pending_tail is not None:
                    pending_tail()
                    pending_tail = None
                if it % NTB == 0 and it > 0:
                    quantize_batch(it // NTB - 1)

                def tail(num_sb=num_sb, xn_sb=xn_sb, it=it):
                    # normalize + relu + output head for tile `it`, deferred
                    # past tile it+1's k-group mults so the Pool-tree wait
                    # doesn't block DVE's queue between tiles.  (HW allows
                    # at most one PSUM input per DVE instruction, so the
                    # P-add and rsb-mult stay two separate stts.)
                    num2_sb = small.tile([128, TN2], bf16, name="num2_sb")
                    nc.vector.scalar_tensor_tensor(
                        out=num2_sb, in0=P_bank[it % 2], scalar=0.0,
                        in1=num_sb, op0=OP.bypass, op1=OP.add)
                    o1p_sb = small.tile([128, TN2], bf16, name="o1p_sb")
                    nc.vector.scalar_tensor_tensor(
                        out=o1p_sb, in0=num2_sb, scalar=0.0,
                        in1=rsb_bank[it % 2], op0=OP.bypass, op1=OP.mult)
                    o1_sb = small.tile([128, TN2], bf16, name="o1_sb")
                    nc.gpsimd.tensor_scalar(
                        out=o1_sb, in0=o1p_sb, scalar1=csb["b3dd"],
                        scalar2=0.0, op0=OP.add, op1=OP.max)
                    out2_ps = P_bank[it % 2]
                    mm(out2_ps, csb["woutddT"], o1_sb, start=True, stop=False)
                    mm(out2_ps, csb["idd"], xn_sb[:, 0, :], start=False,
                       stop=False)
                    mm(out2_ps, csb["boutrow"], csb["ones1"], start=False,
                       stop=True)
                    nc.scalar.copy(
                        out=fin_seg[:, (it % NTB) * TN2:
                                    (it % NTB + 1) * TN2], in_=out2_ps)
                pending_tail = tail

                # ---------- gate chain two tiles ahead ----------
                if build:
                    e_n2 = gate_chain(it + 2, xn_n2, pt80_n2)
                    xn_cur, xn_nxt = xn_nxt, xn_n2
                    e_cur, e_nxt = e_nxt, e_n2
                    ws_cur = ws_nxt
                    feats_cur, pt80_cur = feats_nxt, pt80_nxt
                    feats_nxt, pt80_nxt = feats_n2, pt80_n2
                    if it + 3 < NT:
                        feats_n2, pt80_n2 = feats_n3, pt80_n3
                elif it + 1 < NT:
                    xn_cur = xn_nxt
                    e_cur = e_nxt
                    ws_cur = ws_nxt
                    feats_cur, pt80_cur = feats_nxt, pt80_nxt

            pending_tail()
            quantize_batch(NBATCH - 1)
            nc.sync.dma_start(out=out_d[:, 0:OUTC], in_=u8a)
            nc.sync.dma_start(
                out=out_d[:, OUTC:OUTW].bitcast(f32), in_=scales_sb)

    nc.compile()
    return nc


def _fold_weights(inp):
    """Host-side weight folding -> dict of const arrays (f32)."""
    W0, b0 = inp["W0"], inp["b0"]
    W1, b1 = inp["W1"], inp["b1"]
    W2, b2 = inp["W2"], inp["b2"]
    W3, b3 = inp["W3"], inp["b3"]
    Wp1, Wp2 = inp["Wp1"], inp["Wp2"]
    Ww1, Ww2, bw2 = inp["Ww1"], inp["Ww2"], inp["bw2"]
    Wout, bout = inp["Wout"], inp["bout"]

    Ww1r = Ww1.reshape(GN, CH, K)
    A = Ww1r.sum(axis=2)
    AW1 = A @ W1
    C2 = np.einsum("omj,mc->ocj", Ww1r, W2)
    Dc = -C2.copy()
    Dc[:, :, 0] += AW1
    cb = A @ (b1 - b2)
    v = Wp2 @ np.maximum(Wp1[:, 0], 0.0)
    w1v = np.einsum("omj,m->oj", Ww1r, v)

    m64 = np.arange(CH)

    c = {}
    t = np.zeros((128, 128), np.float32)
    t[0:64, 0:64] = W0.T; t[64:128, 64:128] = W0.T
    c["w0ddT"] = t
    c["b0dd"] = np.concatenate([b0, b0]).reshape(128, 1)
    t = np.zeros((128, 128), np.float32)
    t[0:64, 0:64] = W3.T; t[64:128, 64:128] = W3.T
    c["w3ddT"] = t
    t = np.zeros((128, 10 * 16), np.float32)
    for j in range(K):
        t[0:64, 16 * j:16 * j + 8] = Dc[:, :, j].T
        t[64:128, 16 * j + 8:16 * j + 16] = Dc[:, :, j].T
    c["dls"] = t
    t = np.zeros((K, 16), np.float32)
    for j in range(K):
        t[j, 0:8] = w1v[:, j]
    c["w1vA"] = t
    t = np.zeros((K, 16), np.float32)
    for j in range(K):
        t[j, 8:16] = w1v[:, j]
    c["w1vB"] = t
    c["cbdd"] = np.concatenate([cb, cb]).reshape(16, 1).astype(np.float32)
    # vsel[g*10+k, c + 64h] = v[c] * [g == c % 8]: the per-k gated-ptsn
    # broadcast; contraction over the 80 rows sums over k for free.
    for h, nm in ((0, "vselA"), (1, "vselB")):
        t = np.zeros((80, 128), np.float32)
        for g in range(SP):
            for k in range(K):
                cc = m64[m64 % SP == g]
                t[g * K + k, cc + 64 * h] = v[cc]
        c[nm] = t
    t = np.zeros((16, 80), np.float32); t[0:8, :] = Ww2.T
    c["ww2A"] = t
    t = np.zeros((16, 80), np.float32); t[8:16, :] = Ww2.T
    c["ww2B"] = t
    c["bw2col"] = bw2.reshape(80, 1).astype(np.float32)
    t = np.zeros((80, 16), np.float32)
    for g in range(SP):
        for j in range(K):
            t[g * K + j, g] = 1.0
    c["s8selA"] = t
    t = np.zeros((80, 16), np.float32)
    for g in range(SP):
        for j in range(K):
            t[g * K + j, 8 + g] = 1.0
    c["s8selB"] = t
    # ws replication selectors: block k is [80, 64] with
    # wrep[(c%8)*10 + k, c] = 1 (shared by both point-halves)
    t = np.zeros((80, 640), np.float32)
    for k in range(K):
        for cc in range(64):
            t[(cc % SP) * K + k, 64 * k + cc] = 1.0
    c["wrep"] = t
    t = np.zeros((16, 128), np.float32)
    for h in range(2):
        t[(m64 % SP) + 8 * h, m64 + 64 * h] = 1.0
    c["obcb"] = t
    t = np.zeros((128, 128), np.float32)
    t[0:64, 0:64] = Wout.T; t[64:128, 64:128] = Wout.T
    c["woutddT"] = t
    c["idd"] = np.eye(128, dtype=np.float32)
    c["b3dd"] = np.concatenate([b3, b3]).reshape(128, 1)
    c["boutdd"] = np.concatenate([bout, bout]).reshape(128, 1)
    c["boutrow"] = np.concatenate([bout, bout]).reshape(1, 128)
    c["ones1"] = np.ones((1, TN2), np.float32)
    return c


def make_in_maps(inputs):
    import ml_dtypes
    bf16 = ml_dtypes.bfloat16
    inp = {k: np.ascontiguousarray(np.asarray(v, dtype=np.float32))
           for k, v in inputs.items()}
    consts = _fold_weights(inp)
    cpack_f = np.zeros((128, _F32_W), np.float32)
    for name, (r, c, off) in _F32_LAYOUT.items():
        cpack_f[0:r, off:off + c] = consts[name]
    f8 = ml_dtypes.float8_e4m3
    cpack_8 = np.zeros((128, _F8_W), f8)
    for name, (r, c, off) in _F8_LAYOUT.items():
        cpack_8[0:r, off:off + c] = consts[name].astype(f8)
    cf_bytes = np.frombuffer(cpack_f.tobytes(), np.int8)
    c8_bytes = np.frombuffer(cpack_8.tobytes(), np.int8)
    # host ptsn for all cores at once: [B, N, K]
    cent = inp["cent_pts"]                      # [B, N, 3]
    spt = inp["sm_pts"]                         # [B, 3, N, K]
    ptsn = ((cent.transpose(0, 2, 1)[:, :, :, None] - spt) ** 2).sum(axis=1)
    # k-major pack per batch: [64, NTB, 2, TN2, K] -> [64, NTB, 2, K, TN2],
    # batches concatenated along columns (global tile index = b*NTB + it)
    fpks = []
    for b in range(NBATCH):
        ff = inp["sm_feats"][b].reshape(CH, NTB, 2, TN2, K)
        ff = ff.transpose(0, 1, 2, 4, 3)
        fpks.append(np.concatenate([ff[:, :, 0], ff[:, :, 1]], axis=0)
                    .reshape(128, N * K // 2))
    fpk = np.ascontiguousarray(np.concatenate(fpks, axis=1))
    # int8 quantization, one global MSE-optimal clip (subsampled scan);
    # the scale folds into w0ddT below
    amax = float(np.abs(fpk).max())
    sub = fpk.reshape(-1)[::17]
    best_s, best_mse = None, np.inf
    for frac in (0.68, 0.71, 0.74, 0.77, 0.80, 1.0):
        s = frac * amax / 127.0
        qs = np.clip(np.round(sub / s), -127, 127)
        mse = float(((qs * s - sub) ** 2).mean())
        if mse < best_mse:
            best_s, best_mse = s, mse
    fq = np.clip(np.round(fpk / best_s), -127, 127).astype(np.int8)
    cpack_b = np.zeros((128, _BF16_W), bf16)
    for name, (r, c, off) in _BF16_LAYOUT.items():
        arr = consts[name]
        if name == "w0ddT":
            arr = arr * best_s
        cpack_b[0:r, off:off + c] = arr.astype(bf16)
    # pt80[b*NTB+it, k, h*256+n] = ptsn[b, it, h, n, k]
    pt80 = np.ascontiguousarray(
        ptsn.reshape(NBATCH * NTB, 2, TN2, K).transpose(0, 3, 1, 2)
        .reshape(NT, K, 2 * TN2).astype(bf16))
    blob = np.zeros(TOTAL_BYTES, np.int8)
    blob[OFF_FEATS:OFF_FEATS + FEATS_BYTES] = fq.reshape(-1)
    blob[OFF_PT80:OFF_PT80 + PT80_BYTES] = \
        np.frombuffer(pt80.tobytes(), np.int8)
    blob[OFF_CB:OFF_CB + CB_BYTES] = \
        np.frombuffer(cpack_b.tobytes(), np.int8)
    blob[OFF_CF:OFF_CF + CF_BYTES] = cf_bytes
    blob[OFF_C8:OFF_C8 + C8_BYTES] = c8_bytes
    return [{"blob": blob}]


def _run(inputs, trace=False):
    from concourse.bass_utils import run_bass_kernel_spmd

    if "nc" not in _CACHE:
        _CACHE["nc"] = _build_bass()
    nc = _CACHE["nc"]
    in_maps = make_in_maps(inputs)

    res = run_bass_kernel_spmd(
        nc, in_maps, core_ids=list(range(B)), trace=trace)
    raw = np.asarray(res.results[0]["out"])        # [128, OUTW] uint8
    rsc = np.ascontiguousarray(raw[:, OUTC:OUTW]) \
        .view(np.float32)                          # [128, NBATCH] = 127/amax
    outs = []
    for b in range(NBATCH):
        o = (raw[:, b * (N // 2):(b + 1) * (N // 2)].astype(np.float32)
             - 128.0) / rsc[:, b:b + 1]
        o = o.reshape(2, CH, NTB, TN2)             # [half, c, tile, n]
        outs.append(np.ascontiguousarray(
            o.transpose(1, 2, 0, 3).reshape(CH, N)))
    out = np.stack(outs, axis=0)
    return out, res


def kernel(**inputs) -> np.ndarray:
    out, _ = _run(inputs, trace=False)
    return out

